# revision 47
# baseline (speedup 1.0000x reference)
"""Trainium2 Bass kernel for Ernie4.5-VL vision attention (ragged segments).

Contract: kernel(**inputs) takes the FULL unsharded inputs (keyed as in
setup_inputs()) and returns the FULL [S, D] float32 output.

Strategy
--------
All matmuls run on the PE array in float32r (full-rate fp32, ~1.5e-4 rel
err); everything else is fp32. Attention is computed per segment
(block-diagonal, no masks) in a flash-like streaming form that only ever
materializes transposed score tiles.

Mode A (uniform 4x1024 segments, the common case): 2 head-groups x 4
segments across 8 cores; each core runs a per-head software pipeline with
skew 1:

  iter h: qkv j-tiles (2h, 2h+1)  [PE]  ->  rope head h  [Pool/DVE DMA+mul]
          vaug + attention head h-1      [PE transposes, ACT exp, PE PV]
  tail:   attention head 7, then dense projection (5x128 contraction
          tiles assembled by DMA-repacking the per-head attention output)

Engine budget per head: PE ~15.6us (20 qkv mm + 16 transposes + 32 attn
mm), ACT ~10.4us (4 evac + 16 exp), DVE ~7us (rope muls, recip,
normalize), Pool ~11us (rope staging SWDGE DMAs, one rope mul, vaug
copies, partition broadcasts). PE is the critical engine; everything
else hides behind it.

Mode C (any other cu_seqlens): legacy 8-way head-parallel program, every
core sees all segments.

Host does only O(S*D) glue: input transposes/packing, summing the 2 (or
8) per-token partial projections, and the bias adds.
"""

import os
import sys

import numpy as np

H = 16
HD = 80
BLK = 40  # rotate_half half-width
SCALE = HD ** -0.5
N_CORES = 8
D = 1280
NK = D // 128  # contraction tiles for the qkv matmul
ATTN_STRIDE = 96  # head row pitch in the packed attention output (legacy)
MM_DT_NAME = os.environ.get("KERNEL_MM_DT", "float32r")  # or "float32"
KERNEL_DEBUG = bool(int(os.environ.get("KERNEL_DEBUG", "0")))


def _segments(cu_seqlens, S):
    """Intervals matching reference's searchsorted(cu[1:], i, 'right')."""
    b = np.clip(np.sort(np.asarray(cu_seqlens, dtype=np.int64)[1:5]), 0, S)
    bounds = [0] + list(b) + [S]
    segs = []
    for a, e in zip(bounds[:-1], bounds[1:]):
        if e > a:
            segs.append((int(a), int(e)))
    return segs


def _pack_layout(n_h):
    """Pack per-core qkv dims as 40-row blocks, 3 per 128-row tile (8 pad).

    Each tile holds one v-block at row 0 (PE transpose operands must start
    at a 32-aligned partition) and two q/k blocks at rows 40 and 80.
    Returns pos[(sec, h, half)] = (tile, row) and the number of tiles.
    """
    ntiles = 2 * n_h
    pos = {}
    for h in range(n_h):
        pos[("v", h, 0)] = (2 * h, 0)
        pos[("v", h, 1)] = (2 * h + 1, 0)
        pos[("q", h, 0)] = (2 * h, BLK)
        pos[("q", h, 1)] = (2 * h, 2 * BLK)
        pos[("k", h, 0)] = (2 * h + 1, BLK)
        pos[("k", h, 1)] = (2 * h + 1, 2 * BLK)
    return pos, ntiles


def _pieces(start, length, tile_rows=128):
    """Split global row range [start, start+length) into per-tile pieces."""
    out = []
    off = 0
    while off < length:
        g = start + off
        t, r = g // tile_rows, g % tile_rows
        n = min(tile_rows - r, length - off)
        out.append((t, r, n, off))
        off += n
    return out


def _proj_k_tiles(n_h):
    rows = ATTN_STRIDE * n_h
    kt = [128] * (rows // 128)
    if rows % 128:
        kt.append(rows % 128)
    return kt


def _build_program_a(n_h, S_core):
    """Pipelined SPMD program for mode A (single segment per core).

    Engine-AP partition rules on TRN2 (walrus birverifier): compute-engine
    accesses must start at a 32-aligned partition and must not cross a
    64-boundary unless they start on one; cross-partition data movement
    must go through DMA. The layout choices below all follow from this.
    """
    import concourse.mybir as mybir
    import concourse.tile as tile
    from concourse import bacc
    from concourse.masks import make_identity
    from contextlib import ExitStack

    f32 = mybir.dt.float32
    bf16 = mybir.dt.bfloat16
    mm_dt = getattr(mybir.dt, MM_DT_NAME)
    AF = mybir.ActivationFunctionType

    pos, n_mtiles = _pack_layout(n_h)
    dims_pad = n_mtiles * 128
    VW = 81  # v_aug slot: ones col at 0 (-> denominator on psum row 0), v at 1:81
    n_tt = S_core // 128
    assert S_core % 128 == 0
    BA = 512
    chunks = [(c, min(c + BA, S_core)) for c in range(0, S_core, BA)]
    n_pk = (n_h * HD) // 128  # dense proj contraction tiles
    assert (n_h * HD) % 128 == 0

    nc = bacc.Bacc("TRN2", target_bir_lowering=False, debug=False,
                   enable_asserts=False, num_devices=N_CORES)

    # the two big input streams come in as bf16 (halves HBM traffic; rel
    # err contribution ~4e-3, well under the 2e-2 gate); attention math
    # stays f32r end to end. wqkvT is j-major: tile j's columns contiguous.
    hiddenT = nc.dram_tensor("hiddenT", [128, NK * S_core], bf16,
                             kind="ExternalInput").ap()
    wqkvT = nc.dram_tensor("wqkvT", [128, n_mtiles * NK * 128], bf16,
                           kind="ExternalInput").ap()
    bias2d = nc.dram_tensor("bias2d", [128, n_mtiles], f32,
                            kind="ExternalInput").ap()
    cosD = nc.dram_tensor("cosD", [HD, S_core], mm_dt,
                          kind="ExternalInput").ap()
    # sin2D rows 0:40 hold -sin_lo (they multiply x_hi), rows 40:80 hold
    # +sin_hi (they multiply x_lo); staging swaps the halves of x.
    sin2D = nc.dram_tensor("sin2D", [HD, S_core], mm_dt,
                           kind="ExternalInput").ap()
    wprojP = nc.dram_tensor("wprojP", [128, n_pk * D], bf16,
                            kind="ExternalInput").ap()
    vinitD = nc.dram_tensor("vinitD", [128, n_tt], mm_dt,
                            kind="ExternalInput").ap()
    outT = nc.dram_tensor("outT", [D, S_core], f32, kind="ExternalOutput").ap()

    def r_(ap):
        return ap.bitcast(mm_dt)

    def halves(c0, c1):
        out = []
        q = c0
        while q < c1:
            out.append((q, min(q + 512, c1)))
            q = q + 512
        return out

    hidden3 = hiddenT.rearrange("p (k s) -> p k s", k=NK)
    wp3 = wprojP.rearrange("p (k m) -> p k m", k=n_pk)

    with tile.TileContext(nc) as tc, ExitStack() as ctx:
        persist = ctx.enter_context(tc.tile_pool(name="persist", bufs=1))
        ident = persist.tile([128, 128], f32, tag="ident", name="ident")
        make_identity(nc, ident[:])
        bias_sb = persist.tile([128, n_mtiles], f32, tag="bias", name="bias")
        cos_sb = persist.tile([HD, S_core], mm_dt, tag="cos", name="cos")
        sin2_sb = persist.tile([HD, S_core], mm_dt, tag="sin2", name="sin2")

        psum = ctx.enter_context(tc.tile_pool(name="psum", bufs=1,
                                              space="PSUM"))
        work = ctx.enter_context(tc.tile_pool(name="work", bufs=1))

        # weight stream first (first qkv matmul needs wj0), then hidden
        # trickled per-k so the first qkv tile starts early
        wj_sb = {}

        def emit_wj(j):
            wj_sb[j] = work.tile([128, NK * 128], bf16, tag=f"wj{j % 6}",
                                 name=f"wj{j}", bufs=1)
            nc.sync.dma_start(wj_sb[j][:],
                              wqkvT[:, j * NK * 128:(j + 1) * NK * 128])

        for j in (0, 1):
            emit_wj(j)
        hid_sb = []
        for k in range(NK):
            t = work.tile([128, S_core], bf16, tag=f"hid{k}", name=f"hid{k}")
            hid_sb.append(t)
        for k in range(3):
            nc.sync.dma_start(hid_sb[k][:], hidden3[:, k, :])
        # small persistents after the latency-critical first loads
        nc.sync.dma_start(bias_sb[:], bias2d[:])
        nc.sync.dma_start(cos_sb[:], cosD[:])
        nc.sync.dma_start(sin2_sb[:], sin2D[:])
        for k in range(3, NK):
            nc.sync.dma_start(hid_sb[k][:], hidden3[:, k, :])

        def hid_ap(k, h0, h1):
            return hid_sb[k][:, h0:h1]
        # dense proj weights (loaded mid-pipeline, see head loop)
        wp_sb = []
        for kt in range(n_pk):
            wp_sb.append(work.tile([128, D], bf16, tag=f"wp{kt}",
                                   name=f"wp{kt}"))

        qkv_sb = {}     # j -> tile (ring of 8)
        rot_sb = {}     # (sec, h) -> tile (ring of 4)
        vaug_sb = {}    # h -> tile (ring of 2)
        packed = [work.tile([128, S_core], bf16, tag=f"pk{kt}",
                            name=f"pk{kt}") for kt in range(n_pk)]

        def emit_qkv_half(j, hi):
                if hi == 0:
                    qkv_sb[j] = work.tile([128, S_core], mm_dt,
                                          tag=f"qkv{j % 6}", name=f"qkvT{j}",
                                          bufs=1)
                h0, h1 = halves(0, S_core)[hi]
                hw = h1 - h0
                wj = wj_sb[j]
                ps = psum.tile([128, 512], f32, tag=f"mm{hi % 2}",
                               name="qkvp")
                for k in range(NK):
                    nc.tensor.matmul(
                        ps[:, :hw],
                        wj[:, k * 128:(k + 1) * 128],
                        hid_ap(k, h0, h1),
                        start=(k == 0), stop=(k == NK - 1))
                nc.scalar.activation(qkv_sb[j][:, h0:h1], ps[:, :hw],
                                     AF.Identity,
                                     bias=bias_sb[:, j:j + 1])

        def emit_qkv_j(j):
            for hi in range(len(halves(0, S_core))):
                emit_qkv_half(j, hi)

        def emit_wj_prefetch(h):
            for j in (2 * h, 2 * h + 1):
                if j < n_mtiles and j not in wj_sb:
                    emit_wj(j)

        def emit_rope_sec(h, sec):
            # stage x and swap(x) via SWDGE (Pool) DMA, then
            # rot = x*cos + swap(x)*sin2 on DVE (one mul on Pool)
            for sec in (sec,):
                lo_t, lo_r = pos[(sec, h, 0)]
                hi_t, hi_r = pos[(sec, h, 1)]
                assert hi_t == lo_t and hi_r == lo_r + BLK
                x = qkv_sb[lo_t]
                sa = work.tile([HD, S_core], mm_dt,
                               tag=f"sa{0 if sec == 'q' else 1}", name="sa",
                               bufs=2)
                sb = work.tile([HD, S_core], mm_dt,
                               tag=f"sb{0 if sec == 'q' else 1}", name="sb",
                               bufs=2)
                # sa via SP, sb via Pool: the two staging paths overlap, and
                # mulA (DVE) runs in parallel with mulB (Pool)
                nc.sync.dma_start(sa[0:HD, :], x[lo_r:lo_r + HD, :])
                nc.gpsimd.dma_start(sb[0:BLK, :], x[hi_r:hi_r + BLK, :])
                nc.gpsimd.dma_start(sb[BLK:HD, :], x[lo_r:lo_r + BLK, :])
                rot = work.tile([HD, S_core], mm_dt,
                                tag=f"rot_{sec}{h % 2}", name=f"rot_{sec}{h}",
                                bufs=1)
                rot_sb[(sec, h)] = rot
                nc.vector.tensor_mul(rot[0:HD, :], sa[0:HD, :],
                                     cos_sb[0:HD, :])
                nc.gpsimd.tensor_mul(sb[0:HD, :], sb[0:HD, :],
                                     sin2_sb[0:HD, :])
                nc.vector.tensor_add(rot[0:HD, :], rot[0:HD, :], sb[0:HD, :])

        def emit_rope(h):
            emit_rope_sec(h, "q")
            emit_rope_sec(h, "k")

        GRP = 4  # key tiles transposed per psum tile / copy

        def emit_vaug(h):
            va = work.tile([128, n_tt * VW], mm_dt, tag=f"vaug{h % 2}",
                           name=f"vaug{h}", bufs=1)
            vaug_sb[h] = va
            va3 = va.rearrange("p (t c) -> p t c", c=VW)
            vi3 = vinitD.rearrange("p (t c) -> p t c", c=1)
            nc.gpsimd.dma_start(va3[:, :, 0:1], vi3[:, :, :])
            gi = 0
            gidx = 0
            while gi < n_tt:
                ng = min(GRP, n_tt - gi)
                tp = psum.tile([128, 512], f32, tag=f"x{gidx % 2}", name="tp")
                for x in range(ng):
                    t0 = (gi + x) * 128
                    for half in (0, 1):
                        vt, vr = pos[("v", h, half)]
                        nc.tensor.transpose(
                            tp[:, x * HD + half * BLK:
                               x * HD + (half + 1) * BLK],
                            qkv_sb[vt][0:BLK, t0:t0 + 128].bitcast(f32),
                            ident[:BLK, :BLK])
                src = tp[:, :ng * HD].rearrange("p (t c) -> p t c", c=HD)
                if gidx % 2 == 0:
                    nc.vector.tensor_copy(va3[:, gi:gi + ng, 1:1 + HD],
                                          src[:, :, :])
                else:
                    nc.scalar.activation(va3[:, gi:gi + ng, 1:1 + HD],
                                         src[:, :, :], AF.Identity)
                gi += ng
                gidx += 1

        att_po = {}

        def emit_att_burst(h, ci, ti0, ti1):
            qT = rot_sb[("q", h)]
            kT = rot_sb[("k", h)]
            q0, q1 = chunks[ci]
            qs = q1 - q0
            if ti0 == 0:
                att_po[(h, ci)] = psum.tile([128, BA], f32,
                                            tag=f"po{ci % 2}", name="pv")
            po = att_po[(h, ci)]
            for ti in range(ti0, ti1):
                t0 = ti * 128
                ps = psum.tile([128, BA], f32, tag=f"st{ti % 2}", name="st")
                nc.tensor.matmul(ps[:, :qs], r_(kT[0:HD, t0:t0 + 128]),
                                 r_(qT[0:HD, q0:q1]),
                                 start=True, stop=True)
                pt = work.tile([128, BA], mm_dt, tag="pt", name="pt", bufs=3)
                nc.scalar.activation(pt[:, :qs], ps[:, :qs], AF.Exp)
                nc.tensor.matmul(
                    po[:VW, :qs],
                    r_(vaug_sb[h][:, ti * VW:(ti + 1) * VW]),
                    r_(pt[:, :qs]),
                    start=(ti == 0), stop=(ti == n_tt - 1))

        def emit_att_chunk(h, ci):
            emit_att_burst(h, ci, 0, n_tt)

        def emit_att_norm(h, ci):
            (q0, q1) = chunks[ci]
            qs = q1 - q0
            po = att_po.pop((h, ci))
            rc = work.tile([1, BA], f32, tag="rc", name="rc", bufs=2)
            nc.vector.reciprocal(rc[0:1, :qs], po[0:1, :qs])
            bc = work.tile([VW, BA], mm_dt, tag="bc", name="bc", bufs=2)
            nc.gpsimd.partition_broadcast(bc[0:VW, :qs],
                                          rc[0:1, :qs].bitcast(mm_dt))
            ast = work.tile([VW, BA], bf16, tag="ast", name="ast", bufs=3)
            nc.vector.tensor_mul(ast[0:VW, :qs], po[0:VW, :qs],
                                 bc[0:VW, :qs])
            for (t, r, n, off) in _pieces(HD * h, HD):
                nc.sync.dma_start(packed[t][r:r + n, q0:q1],
                                  ast[1 + off:1 + off + n, :qs])

        def emit_proj_chunk(ci):
            c0, c1 = chunks[ci]
            cs = c1 - c0
            for j in range(D // 128):
                ps = psum.tile([128, 512], f32, tag=f"mm{j % 2}", name="pj")
                for kt in range(n_pk):
                    nc.tensor.matmul(
                        ps[:, :cs],
                        wp_sb[kt][:, j * 128:(j + 1) * 128],
                        packed[kt][:, c0:c1],
                        start=(kt == 0), stop=(kt == n_pk - 1))
                ob = work.tile([128, BA], f32, tag="ob", name="ob", bufs=3)
                if j % 2 == 0:
                    nc.vector.tensor_copy(ob[:, :cs], ps[:, :cs])
                else:
                    nc.scalar.activation(ob[:, :cs], ps[:, :cs], AF.Identity)
                nc.scalar.dma_start(outT[j * 128:(j + 1) * 128, c0:c1],
                                    ob[:, :cs])

        # ---- pipeline: attention skewed one head behind qkv ----
        emit_qkv_j(0)
        emit_qkv_j(1)
        emit_wj_prefetch(1)
        emit_wj_prefetch(2)
        emit_rope(0)
        half_tt = n_tt // 2
        for h in range(1, n_h):
            # interleave attention bursts of head h-1 between qkv half
            # tiles of head h: the exps then spread over the whole cycle
            # instead of piling into the short attention window
            emit_vaug(h - 1)
            emit_wj_prefetch(h + 2)
            if h == 3:  # proj weights, needed only at the tail
                for kt in range(n_pk):
                    nc.sync.dma_start(wp_sb[kt][:], wp3[:, kt, :])
            emit_qkv_half(2 * h, 0)
            emit_att_burst(h - 1, 0, 0, half_tt)
            emit_qkv_half(2 * h, 1)
            emit_rope_sec(h, "q")
            emit_att_burst(h - 1, 0, half_tt, n_tt)
            emit_att_norm(h - 1, 0)
            emit_qkv_half(2 * h + 1, 0)
            if h < n_h - 1:
                emit_att_burst(h - 1, 1, 0, half_tt)
                emit_qkv_half(2 * h + 1, 1)
                emit_rope_sec(h, "k")
                emit_att_burst(h - 1, 1, half_tt, n_tt)
            else:
                # last head: issue rope-k as early as possible and keep all
                # of att(h-1, c1) after it as PE cover for the rope chain
                emit_qkv_half(2 * h + 1, 1)
                emit_rope_sec(h, "k")
                emit_att_burst(h - 1, 1, 0, n_tt)
            emit_att_norm(h - 1, 1)
        # tail: head 7 chunk by chunk, hiding each norm+repack chain under
        # other PE work (the other chunk's attention / the projection)
        h7 = n_h - 1
        emit_vaug(h7)
        emit_att_chunk(h7, 0)
        emit_att_norm(h7, 0)
        emit_att_chunk(h7, 1)
        emit_proj_chunk(0)
        emit_att_norm(h7, 1)
        emit_proj_chunk(1)

    nc.compile()
    return nc


def _build_program(n_h, S_core, segs_local, resident_hidden):
    """Legacy SPMD program (mode C fallback). Same structure for every core."""
    import concourse.mybir as mybir
    import concourse.tile as tile
    from concourse import bacc
    from concourse.masks import make_identity
    from contextlib import ExitStack

    f32 = mybir.dt.float32
    mm_dt = getattr(mybir.dt, MM_DT_NAME)
    AF = mybir.ActivationFunctionType

    k_proj = n_h
    pos, n_mtiles = _pack_layout(n_h)
    dims_pad = n_mtiles * 128
    VW = 97  # v_aug slot width: 80 v dims + 16 zero pad + ones col at 96

    # global key-tile list: (seg_idx, t0, t1)
    t_tiles = []
    for si, (a, e) in enumerate(segs_local):
        t = a
        while t < e:
            t_tiles.append((si, t, min(t + 128, e)))
            t += 128
    n_tt = len(t_tiles)

    nc = bacc.Bacc("TRN2", target_bir_lowering=False, debug=False,
                   enable_asserts=False, num_devices=N_CORES)

    # host supplies hiddenT/wqkvT pre-tiled into 128-partition-major layout
    hiddenT = nc.dram_tensor("hiddenT", [128, NK * S_core], mm_dt,
                             kind="ExternalInput").ap()
    wqkvT = nc.dram_tensor("wqkvT", [128, NK * dims_pad], mm_dt,
                           kind="ExternalInput").ap()
    bias2d = nc.dram_tensor("bias2d", [128, n_mtiles], f32,
                            kind="ExternalInput").ap()
    # cosP/sin2P are host-packed [128, S]: rows 0:40 and 64:104 hold the
    # lo/hi rope coefficients, all other rows zero (zeroes the junk rows
    # of the rotated q/k so the K=104 score matmuls see exact zeros).
    cosP = nc.dram_tensor("cosP", [128, S_core], mm_dt,
                          kind="ExternalInput").ap()
    sin2P = nc.dram_tensor("sin2P", [128, S_core], mm_dt,
                           kind="ExternalInput").ap()
    wprojT = nc.dram_tensor("wprojT", [n_h * HD, D], mm_dt,
                            kind="ExternalInput").ap()
    # per-key-tile v_aug tail init: 16 zero pad cols + ones col (f32r memset
    # fails walrus codegen, so this comes in via DMA)
    vinit = nc.dram_tensor("vinit", [128, n_tt * (VW - HD)], mm_dt,
                           kind="ExternalInput").ap()
    outT = nc.dram_tensor("outT", [D, S_core], f32, kind="ExternalOutput").ap()

    def r_(ap):
        return ap.bitcast(mm_dt)

    BC = 1024  # psum tile width (2 banks); matmuls stream <=512
    big_chunks = [(c, min(c + BC, S_core)) for c in range(0, S_core, BC)]

    def halves(c0, c1):
        out = []
        q = c0
        while q < c1:
            out.append((q, min(q + 512, c1)))
            q = q + 512
        return out

    with tile.TileContext(nc) as tc, ExitStack() as ctx:
        persist = ctx.enter_context(tc.tile_pool(name="persist", bufs=1))
        ident = persist.tile([128, 128], f32, tag="ident", name="ident")
        make_identity(nc, ident[:])
        bias_sb = persist.tile([128, n_mtiles], f32, tag="bias", name="bias")
        nc.sync.dma_start(bias_sb[:], bias2d[:])

        psum_all_cm = tc.tile_pool(name="psum_all", bufs=1, space="PSUM")
        psum_all = psum_all_cm.__enter__()
        qkv_pool = ctx.enter_context(tc.tile_pool(name="big", bufs=1))
        qkv_sb = [qkv_pool.tile([128, S_core], mm_dt, tag=f"qkvT{j}",
                                name=f"qkvT{j}") for j in range(n_mtiles)]
        rot_cm = tc.tile_pool(name="rot", bufs=1)
        rv = rot_cm.__enter__()
        rot_sb = {}
        for h in range(n_h):
            for sec in ("q", "k"):
                rot_sb[(sec, h)] = rv.tile([128, S_core], mm_dt,
                                           tag=f"rot_{sec}{h}",
                                           name=f"rot_{sec}{h}")
        RC = 1024
        rope_cm = tc.tile_pool(name="rope_scr", bufs=2)
        rope_scr = rope_cm.__enter__()

        # ------------ phase 1: qkvT = Wpack @ hidden.T --------------
        with ExitStack() as p1:
            hidden3 = hiddenT.rearrange("p (k s) -> p k s", k=NK)
            w3 = wqkvT.rearrange("p (k m) -> p k m", k=NK)
            # k-outer streaming: two psum slots hold four j-streams
            # (columns 0:512 and 512:1024), hidden tiles are tiny
            w_pool = p1.enter_context(tc.tile_pool(name="wres", bufs=1))
            w_sb = [w_pool.tile([128, dims_pad], mm_dt, tag=f"w{k}",
                                name=f"w{k}") for k in range(NK)]
            for k in range(NK):
                nc.sync.dma_start(w_sb[k][:], w3[:, k, :])
            assert n_mtiles == 4
            hid_pool = p1.enter_context(tc.tile_pool(name="hidstream",
                                                     bufs=3))
            for (h0, h1) in halves(0, S_core):
                hw = h1 - h0
                ps01 = psum_all.tile([128, BC], f32, tag="t0", name="ps01")
                ps23 = psum_all.tile([128, BC], f32, tag="t1", name="ps23")
                pj_of = lambda j: (ps01 if j < 2 else ps23,
                                   (j % 2) * 512)
                for k in range(NK):
                    ht = hid_pool.tile([128, 512], mm_dt, tag="hidc",
                                       name="hidc")
                    nc.sync.dma_start(ht[:, :hw], hidden3[:, k, h0:h1])
                    for j in range(n_mtiles):
                        psj, co = pj_of(j)
                        nc.tensor.matmul(
                            psj[:, co:co + hw],
                            r_(w_sb[k][:, j * 128:(j + 1) * 128]),
                            r_(ht[:, :hw]),
                            start=(k == 0), stop=(k == NK - 1))
                for j in range(n_mtiles):
                    psj, co = pj_of(j)
                    nc.scalar.activation(qkv_sb[j][:, h0:h1],
                                         psj[:, co:co + hw], AF.Identity,
                                         bias=bias_sb[:, j:j + 1])

        psum_all_cm.__exit__(None, None, None)
        ps_att = ctx.enter_context(tc.tile_pool(name="ps_att", bufs=1,
                                                space="PSUM"))

        # ------------ phase 2: RoPE --------------------------------
        stg = {}
        for nm in ("sa0", "sa1", "sb0", "sb1"):
            stg[nm] = rope_scr.tile([128, RC], mm_dt, tag=nm, name=nm, bufs=1)
        pair_i = 0
        for ci, f0 in enumerate(range(0, S_core, RC)):
            f1 = min(f0 + RC, S_core)
            fs = f1 - f0
            cos_sb = rope_scr.tile([128, RC], mm_dt, tag="cos", name="cos",
                                   bufs=1)
            sin_sb = rope_scr.tile([128, RC], mm_dt, tag="sin", name="sin",
                                   bufs=1)
            nc.scalar.dma_start(cos_sb[:, :fs], cosP[:, f0:f1])
            nc.scalar.dma_start(sin_sb[:, :fs], sin2P[:, f0:f1])
            if ci == 0:
                for nm in stg:
                    nc.scalar.dma_start(stg[nm][BLK:64, :], cos_sb[BLK:64, :])
            for h in range(n_h):
                for sec in ("q", "k"):
                    lo_t, lo_r = pos[(sec, h, 0)]
                    hi_t, hi_r = pos[(sec, h, 1)]
                    assert hi_t == lo_t and hi_r == lo_r + BLK
                    x = qkv_sb[lo_t]
                    dst = rot_sb[(sec, h)]
                    stga = stg[f"sa{pair_i % 2}"]
                    stgb = stg[f"sb{pair_i % 2}"]
                    nc.scalar.dma_start(stga[0:BLK, :fs],
                                        x[lo_r:lo_r + BLK, f0:f1])
                    nc.scalar.dma_start(stga[64:64 + BLK, :fs],
                                        x[hi_r:hi_r + BLK, f0:f1])
                    nc.scalar.dma_start(stgb[0:BLK, :fs],
                                        x[hi_r:hi_r + BLK, f0:f1])
                    nc.scalar.dma_start(stgb[64:64 + BLK, :fs],
                                        x[lo_r:lo_r + BLK, f0:f1])
                    nc.vector.tensor_mul(dst[0:104, f0:f1], stga[0:104, :fs],
                                         cos_sb[0:104, :fs])
                    eng = nc.gpsimd if pair_i % 2 == 0 else nc.vector
                    eng.tensor_mul(stgb[0:104, :fs], stgb[0:104, :fs],
                                   sin_sb[0:104, :fs])
                    nc.vector.tensor_add(dst[0:104, f0:f1], dst[0:104, f0:f1],
                                         stgb[0:104, :fs])
                    pair_i += 1
        rope_cm.__exit__(None, None, None)

        vaug_cm = tc.tile_pool(name="vaug", bufs=1)
        vaug_pool = vaug_cm.__enter__()
        vaug_sb = [vaug_pool.tile([128, n_tt * VW], mm_dt, tag=f"vaug{h}",
                                  name=f"vaug{h}") for h in range(n_h)]
        vinit3 = vinit.rearrange("p (t c) -> p t c", c=VW - HD)
        for h in range(n_h):
            nc.sync.dma_start(
                vaug_sb[h].rearrange("p (t c) -> p t c", c=VW)[:, :, HD:VW],
                vinit3[:, :, :])
        GRP = 4  # key tiles transposed per psum tile / copy (1 psum bank)

        def emit_vaug(h):
            gi = 0
            while gi < n_tt:
                hi_g = min(gi + GRP, n_tt)
                if all(t_tiles[g][2] - t_tiles[g][1] == 128
                       for g in range(gi, hi_g)):
                    grp = list(range(gi, hi_g))
                else:
                    grp = [gi]
                ng = len(grp)
                tp = ps_att.tile([128, GRP * HD], f32, tag="tp", name="tp")
                for x, g in enumerate(grp):
                    si, t0, t1 = t_tiles[g]
                    sz = t1 - t0
                    for half in (0, 1):
                        vt, vr = pos[("v", h, half)]
                        nc.tensor.transpose(
                            tp[:sz, x * HD + half * BLK:
                               x * HD + (half + 1) * BLK],
                            qkv_sb[vt][0:BLK, t0:t1].bitcast(f32),
                            ident[:BLK, :BLK])
                sz0 = t_tiles[grp[0]][2] - t_tiles[grp[0]][1]
                dst = vaug_sb[h].rearrange("p (t c) -> p t c", c=VW)
                src_ap = tp.rearrange("p (t c) -> p t c", c=HD)
                if h % 2 == 0:
                    nc.vector.tensor_copy(dst[:sz0, grp[0]:grp[0] + ng, 0:HD],
                                          src_ap[:sz0, 0:ng, :])
                else:
                    nc.scalar.activation(dst[:sz0, grp[0]:grp[0] + ng, 0:HD],
                                         src_ap[:sz0, 0:ng, :], AF.Identity)
                gi += ng

        # ------------ phase 4: attention ----------------------------
        attn_sb = [qkv_pool.tile([128, S_core], mm_dt, tag=f"qkvT{h}",
                                 name=f"attnT{h}") for h in range(n_h)]

        seg_ttiles = {}
        for ti, (si, t0, t1) in enumerate(t_tiles):
            seg_ttiles.setdefault(si, []).append((ti, t0, t1))

        BA = 512  # attention query-chunk width (1-bank psum slots)
        with ExitStack() as p4:
            pt_pool = p4.enter_context(tc.tile_pool(name="pt", bufs=3))
            nrm_pool = p4.enter_context(tc.tile_pool(name="nrm", bufs=2))
            unit_box = [0]

            def emit_attention(h, si, a, e):
                qT = rot_sb[("q", h)]
                kT = rot_sb[("k", h)]
                q = a
                while q < e:
                    q0, q1 = q, min(q + BA, e)
                    qs = q1 - q0
                    po = ps_att.tile([128, BA], f32,
                                     tag=f"po{unit_box[0] % 2}", name="pv")
                    tts = seg_ttiles[si]
                    for idx, (ti, t0, t1) in enumerate(tts):
                        sz = t1 - t0
                        ps = ps_att.tile([128, BA], f32, tag=f"st{idx % 2}",
                                         name="st")
                        nc.tensor.matmul(ps[:sz, :qs], r_(kT[0:104, t0:t1]),
                                         r_(qT[0:104, q0:q1]),
                                         start=True, stop=True)
                        pt = pt_pool.tile([128, BA], mm_dt, tag="pt", name="pt")
                        nc.scalar.activation(pt[:sz, :qs], ps[:sz, :qs], AF.Exp)
                        nc.tensor.matmul(
                            po[:VW, :qs],
                            r_(vaug_sb[h][:sz, ti * VW:(ti + 1) * VW]),
                            r_(pt[:sz, :qs]),
                            start=(idx == 0), stop=(idx == len(tts) - 1))
                    # partition_broadcast ucode reads physical partition 0,
                    # so shift the denominator row 96 -> 0 via DMA
                    rc = nrm_pool.tile([128, BA], f32, tag="rc", name="rc")
                    nc.vector.tensor_copy(rc[96:97, :qs], po[96:97, :qs])
                    nc.sync.dma_start(rc[0:1, :qs], rc[96:97, :qs])
                    nc.vector.reciprocal(rc[0:1, :qs], rc[0:1, :qs])
                    bc = nrm_pool.tile([128, BA], mm_dt, tag="bc", name="bc")
                    nc.gpsimd.partition_broadcast(
                        bc[0:HD, :qs], rc[0:1, :qs].bitcast(mm_dt))
                    nc.vector.tensor_mul(attn_sb[h][0:HD, q0:q1],
                                         po[0:HD, :qs], bc[0:HD, :qs])
                    unit_box[0] += 1
                    q = q1

            for h in range(n_h):
                emit_vaug(h)
            for si, (a, e) in enumerate(segs_local):
                for h in range(n_h):
                    emit_attention(h, si, a, e)

        vaug_cm.__exit__(None, None, None)
        rot_cm.__exit__(None, None, None)

        # ------------ phase 5: projection partial -------------------
        with ExitStack() as p5:
            wp_pool = p5.enter_context(tc.tile_pool(name="wp", bufs=1))
            wp_sb = []
            for kt in range(k_proj):
                t = wp_pool.tile([HD, D], mm_dt, tag=f"wp{kt}", name=f"wp{kt}")
                nc.sync.dma_start(t[:], wprojT[kt * HD:(kt + 1) * HD, :])
                wp_sb.append(t)
            out_pool = p5.enter_context(tc.tile_pool(name="outsb", bufs=3))
            for (c0, c1) in big_chunks:
                cs = c1 - c0
                for j in range(D // 128):
                    ob = out_pool.tile([128, BC], f32, tag="ob", name="ob")
                    for (h0, h1) in halves(c0, c1):
                        ps = ps_att.tile([128, 512], f32, tag=f"st{j % 2}",
                                         name="pj")
                        for kt in range(k_proj):
                            nc.tensor.matmul(
                                ps[:, :h1 - h0],
                                r_(wp_sb[kt][:, j * 128:(j + 1) * 128]),
                                r_(attn_sb[kt][0:HD, h0:h1]),
                                start=(kt == 0), stop=(kt == k_proj - 1))
                        if j % 2 == 0:
                            nc.vector.tensor_copy(ob[:, h0 - c0:h1 - c0],
                                                  ps[:, :h1 - h0])
                        else:
                            nc.scalar.activation(ob[:, h0 - c0:h1 - c0],
                                                 ps[:, :h1 - h0], AF.Identity)
                    nc.sync.dma_start(outT[j * 128:(j + 1) * 128, c0:c1],
                                      ob[:, :cs])

    nc.compile()
    return nc


def _pack_w(Wqkv, bqkv, heads, n_h, jmajor=False):
    """Per-core packed qkv weights (q rows pre-scaled).

    Returns wqkvT_tiled [128, NK*dims_pad] (k-major blocks of [128,
    dims_pad], or j-major [128, n_mtiles*NK*128] when jmajor) and bias2d
    [128, n_mtiles]."""
    pos, n_mtiles = _pack_layout(n_h)
    dims_pad = n_mtiles * 128
    W = np.zeros((dims_pad, D), np.float32)
    b = np.zeros((dims_pad,), np.float32)
    sec_off = {"q": 0, "k": D, "v": 2 * D}
    for i, h in enumerate(heads):
        for sec in ("q", "k", "v"):
            for half in (0, 1):
                t, r = pos[(sec, i, half)]
                src = sec_off[sec] + h * HD + half * BLK
                w = Wqkv[src:src + BLK, :]
                bb = bqkv[src:src + BLK]
                if sec == "q":
                    w = w * SCALE
                    bb = bb * SCALE
                W[t * 128 + r:t * 128 + r + BLK] = w
                b[t * 128 + r:t * 128 + r + BLK] = bb
    WT = np.ascontiguousarray(W.T)  # [D = NK*128, dims_pad = n_mtiles*128]
    if jmajor:
        w_tiled = np.ascontiguousarray(
            WT.reshape(NK, 128, n_mtiles, 128).transpose(1, 2, 0, 3)
            .reshape(128, n_mtiles * NK * 128))
    else:
        w_tiled = _tile_rows(WT)
    bias2d = np.ascontiguousarray(b.reshape(n_mtiles, 128).T)
    return w_tiled, bias2d


def _tile_rows(x):
    """[R, C] with R = nk*128 -> [128, nk*C] k-major tiling."""
    R, C = x.shape
    nk = R // 128
    return np.ascontiguousarray(
        x.reshape(nk, 128, C).transpose(1, 0, 2).reshape(128, nk * C))


def _pack_wproj(Wproj, heads):
    """Rows of Wproj.T for this core's head dims, stacked per head."""
    W = np.zeros((len(heads) * HD, Wproj.shape[0]), np.float32)
    for i, h in enumerate(heads):
        W[i * HD:(i + 1) * HD] = Wproj[:, h * HD:(h + 1) * HD].T
    return W


def _pack_cos_sin(cos, sin):
    """cosP/sin2P [128, S]: lo coeffs at rows 0:40, hi at 64:104, rest 0.

    sin2P row signs match rot = x*cosP + swap(x)*sin2P: lo rows hold
    -sin_lo (they multiply x_hi), hi rows hold +sin_hi (they multiply x_lo).
    """
    S = cos.shape[0]
    cosP = np.zeros((128, S), np.float32)
    sinP = np.zeros((128, S), np.float32)
    cosP[0:BLK] = cos.T[0:BLK]
    cosP[64:64 + BLK] = cos.T[BLK:HD]
    sinP[0:BLK] = -sin.T[0:BLK]
    sinP[64:64 + BLK] = sin.T[BLK:HD]
    return cosP, sinP


def _pack_cos_sin_dense(cos, sin):
    """Dense [80, S] rope coefficients for mode A.

    sin2D row signs match rot = x*cos + swap(x)*sin2D: rows 0:40 hold
    -sin_lo (they multiply x_hi), rows 40:80 hold +sin_hi (x_lo)."""
    cosT = np.ascontiguousarray(cos.T.astype(np.float32))
    sinT = sin.T.astype(np.float32)
    sin2 = np.concatenate([-sinT[0:BLK], sinT[BLK:HD]], axis=0)
    return cosT, np.ascontiguousarray(sin2)


_CACHE = {}


def kernel(hidden_states, cos, sin, Wqkv, bqkv, Wproj, bproj, cu_seqlens):
    sys.path.insert(0, "/opt/trn_rl_repo")
    from concourse import bass_utils

    hidden_states = np.asarray(hidden_states, np.float32)
    cos = np.asarray(cos, np.float32)
    sin = np.asarray(sin, np.float32)
    Wqkv = np.asarray(Wqkv, np.float32)
    bqkv = np.asarray(bqkv, np.float32)
    Wproj = np.asarray(Wproj, np.float32)
    bproj = np.asarray(bproj, np.float32)

    S, D_ = hidden_states.shape
    assert D_ == D
    segs = _segments(cu_seqlens, S)
    uniform = (S % 4 == 0) and segs == [(i * S // 4, (i + 1) * S // 4)
                                        for i in range(4)]

    hiddenT = np.ascontiguousarray(hidden_states.T)

    if uniform:
        # mode A: 2 head-groups x 4 segments, pipelined program
        n_h, S_core = H // 2, S // 4
        key = ("A", S)
        if key not in _CACHE:
            _CACHE[key] = _build_program_a(n_h, S_core)
        nc = _CACHE[key]
        import ml_dtypes
        bf = ml_dtypes.bfloat16
        cosD, sin2D = _pack_cos_sin_dense(cos, sin)
        n_tt = S_core // 128
        vinitD = np.ones((128, n_tt), np.float32)
        in_maps = []
        meta = []
        for g in range(2):
            heads = list(range(g * n_h, (g + 1) * n_h))
            wt, b2 = _pack_w(Wqkv, bqkv, heads, n_h, jmajor=True)
            wt = wt.astype(bf)
            wprojP = _tile_rows(_pack_wproj(Wproj, heads)).astype(bf)
            for s in range(4):
                sl = slice(s * S_core, (s + 1) * S_core)
                in_maps.append({
                    "hiddenT": _tile_rows(hiddenT[:, sl]).astype(bf),
                    "wqkvT": wt,
                    "bias2d": b2,
                    "cosD": np.ascontiguousarray(cosD[:, sl]),
                    "sin2D": np.ascontiguousarray(sin2D[:, sl]),
                    "wprojP": wprojP,
                    "vinitD": vinitD,
                })
                meta.append((g, s))
        res = bass_utils.run_bass_kernel_spmd(nc, in_maps,
                                              core_ids=list(range(N_CORES)))
        out = np.zeros((D, S), np.float32)
        for c, (g, s) in enumerate(meta):
            out[:, s * S_core:(s + 1) * S_core] += res.results[c]["outT"]
    else:
        # mode C: 8-way head parallel, full sequence per core
        n_h, S_core = H // N_CORES, S
        key = ("C", S, tuple(np.asarray(cu_seqlens).tolist()))
        if key not in _CACHE:
            _CACHE[key] = _build_program(n_h, S_core, segs,
                                         resident_hidden=False)
        nc = _CACHE[key]
        cosP, sin2P = _pack_cos_sin(cos, sin)

        def _vinit(segs_local):
            n_tt = sum(-(-(e - a) // 128) for a, e in segs_local)
            v = np.zeros((128, n_tt, 17), np.float32)
            v[:, :, 16] = 1.0
            return np.ascontiguousarray(v.reshape(128, n_tt * 17))

        vinit = _vinit(segs)
        hid_tiled = _tile_rows(hiddenT)
        in_maps = []
        for c in range(N_CORES):
            heads = list(range(c * n_h, (c + 1) * n_h))
            wt, b2 = _pack_w(Wqkv, bqkv, heads, n_h)
            in_maps.append({
                "hiddenT": hid_tiled,
                "wqkvT": wt,
                "bias2d": b2,
                "cosP": cosP,
                "sin2P": sin2P,
                "wprojT": _pack_wproj(Wproj, heads),
                "vinit": vinit,
            })
        res = bass_utils.run_bass_kernel_spmd(nc, in_maps,
                                              core_ids=list(range(N_CORES)))
        out = np.zeros((D, S), np.float32)
        for c in range(N_CORES):
            out += res.results[c]["outT"]

    return np.ascontiguousarray(out.T) + bproj[None, :]


# revision 48
# speedup vs baseline: 1.0179x; 1.0179x over previous
"""Trainium2 Bass kernel for Ernie4.5-VL vision attention (ragged segments).

Contract: kernel(**inputs) takes the FULL unsharded inputs (keyed as in
setup_inputs()) and returns the FULL [S, D] float32 output.

Strategy
--------
All matmuls run on the PE array in float32r (full-rate fp32, ~1.5e-4 rel
err); everything else is fp32. Attention is computed per segment
(block-diagonal, no masks) in a flash-like streaming form that only ever
materializes transposed score tiles.

Mode A (uniform 4x1024 segments, the common case): 2 head-groups x 4
segments across 8 cores; each core runs a per-head software pipeline with
skew 1:

  iter h: qkv j-tiles (2h, 2h+1)  [PE]  ->  rope head h  [Pool/DVE DMA+mul]
          vaug + attention head h-1      [PE transposes, ACT exp, PE PV]
  tail:   attention head 7, then dense projection (5x128 contraction
          tiles assembled by DMA-repacking the per-head attention output)

Engine budget per head: PE ~15.6us (20 qkv mm + 16 transposes + 32 attn
mm), ACT ~10.4us (4 evac + 16 exp), DVE ~7us (rope muls, recip,
normalize), Pool ~11us (rope staging SWDGE DMAs, one rope mul, vaug
copies, partition broadcasts). PE is the critical engine; everything
else hides behind it.

Mode C (any other cu_seqlens): legacy 8-way head-parallel program, every
core sees all segments.

Host does only O(S*D) glue: input transposes/packing, summing the 2 (or
8) per-token partial projections, and the bias adds.
"""

import os
import sys

import numpy as np

H = 16
HD = 80
BLK = 40  # rotate_half half-width
SCALE = HD ** -0.5
N_CORES = 8
D = 1280
NK = D // 128  # contraction tiles for the qkv matmul
ATTN_STRIDE = 96  # head row pitch in the packed attention output (legacy)
MM_DT_NAME = os.environ.get("KERNEL_MM_DT", "float32r")  # or "float32"
KERNEL_DEBUG = bool(int(os.environ.get("KERNEL_DEBUG", "0")))


def _segments(cu_seqlens, S):
    """Intervals matching reference's searchsorted(cu[1:], i, 'right')."""
    b = np.clip(np.sort(np.asarray(cu_seqlens, dtype=np.int64)[1:5]), 0, S)
    bounds = [0] + list(b) + [S]
    segs = []
    for a, e in zip(bounds[:-1], bounds[1:]):
        if e > a:
            segs.append((int(a), int(e)))
    return segs


def _pack_layout(n_h):
    """Pack per-core qkv dims as 40-row blocks, 3 per 128-row tile (8 pad).

    Each tile holds one v-block at row 0 (PE transpose operands must start
    at a 32-aligned partition) and two q/k blocks at rows 40 and 80.
    Returns pos[(sec, h, half)] = (tile, row) and the number of tiles.
    """
    ntiles = 2 * n_h
    pos = {}
    for h in range(n_h):
        pos[("v", h, 0)] = (2 * h, 0)
        pos[("v", h, 1)] = (2 * h + 1, 0)
        pos[("q", h, 0)] = (2 * h, BLK)
        pos[("q", h, 1)] = (2 * h, 2 * BLK)
        pos[("k", h, 0)] = (2 * h + 1, BLK)
        pos[("k", h, 1)] = (2 * h + 1, 2 * BLK)
    return pos, ntiles


def _pieces(start, length, tile_rows=128):
    """Split global row range [start, start+length) into per-tile pieces."""
    out = []
    off = 0
    while off < length:
        g = start + off
        t, r = g // tile_rows, g % tile_rows
        n = min(tile_rows - r, length - off)
        out.append((t, r, n, off))
        off += n
    return out


def _proj_k_tiles(n_h):
    rows = ATTN_STRIDE * n_h
    kt = [128] * (rows // 128)
    if rows % 128:
        kt.append(rows % 128)
    return kt


def _build_program_a(n_h, S_core):
    """Pipelined SPMD program for mode A (single segment per core).

    Engine-AP partition rules on TRN2 (walrus birverifier): compute-engine
    accesses must start at a 32-aligned partition and must not cross a
    64-boundary unless they start on one; cross-partition data movement
    must go through DMA. The layout choices below all follow from this.
    """
    import concourse.mybir as mybir
    import concourse.tile as tile
    from concourse import bacc
    from concourse.masks import make_identity
    from contextlib import ExitStack

    f32 = mybir.dt.float32
    bf16 = mybir.dt.bfloat16
    mm_dt = getattr(mybir.dt, MM_DT_NAME)
    AF = mybir.ActivationFunctionType

    pos, n_mtiles = _pack_layout(n_h)
    dims_pad = n_mtiles * 128
    VW = 81  # v_aug slot: ones col at 0 (-> denominator on psum row 0), v at 1:81
    n_tt = S_core // 128
    assert S_core % 128 == 0
    BA = 512
    chunks = [(c, min(c + BA, S_core)) for c in range(0, S_core, BA)]
    n_pk = (n_h * HD) // 128  # dense proj contraction tiles
    assert (n_h * HD) % 128 == 0

    nc = bacc.Bacc("TRN2", target_bir_lowering=False, debug=False,
                   enable_asserts=False, num_devices=N_CORES)

    # the two big input streams come in as bf16 (halves HBM traffic; rel
    # err contribution ~4e-3, well under the 2e-2 gate); attention math
    # stays f32r end to end. wqkvT is j-major: tile j's columns contiguous.
    hiddenT = nc.dram_tensor("hiddenT", [128, NK * S_core], bf16,
                             kind="ExternalInput").ap()
    wqkvT = nc.dram_tensor("wqkvT", [128, n_mtiles * NK * 128], bf16,
                           kind="ExternalInput").ap()
    bias2d = nc.dram_tensor("bias2d", [128, n_mtiles], f32,
                            kind="ExternalInput").ap()
    cosD = nc.dram_tensor("cosD", [HD, S_core], mm_dt,
                          kind="ExternalInput").ap()
    # sin2D rows 0:40 hold -sin_lo (they multiply x_hi), rows 40:80 hold
    # +sin_hi (they multiply x_lo); staging swaps the halves of x.
    sin2D = nc.dram_tensor("sin2D", [HD, S_core], mm_dt,
                           kind="ExternalInput").ap()
    wprojP = nc.dram_tensor("wprojP", [128, n_pk * D], bf16,
                            kind="ExternalInput").ap()
    vinitD = nc.dram_tensor("vinitD", [128, n_tt], mm_dt,
                            kind="ExternalInput").ap()
    outT = nc.dram_tensor("outT", [D, S_core], f32, kind="ExternalOutput").ap()

    def r_(ap):
        return ap.bitcast(mm_dt)

    def halves(c0, c1):
        out = []
        q = c0
        while q < c1:
            out.append((q, min(q + 512, c1)))
            q = q + 512
        return out

    hidden3 = hiddenT.rearrange("p (k s) -> p k s", k=NK)
    wp3 = wprojP.rearrange("p (k m) -> p k m", k=n_pk)

    with tile.TileContext(nc) as tc, ExitStack() as ctx:
        persist = ctx.enter_context(tc.tile_pool(name="persist", bufs=1))
        ident = persist.tile([128, 128], f32, tag="ident", name="ident")
        make_identity(nc, ident[:])
        bias_sb = persist.tile([128, n_mtiles], f32, tag="bias", name="bias")
        cos_sb = persist.tile([HD, S_core], mm_dt, tag="cos", name="cos")
        sin2_sb = persist.tile([HD, S_core], mm_dt, tag="sin2", name="sin2")

        psum = ctx.enter_context(tc.tile_pool(name="psum", bufs=1,
                                              space="PSUM"))
        work = ctx.enter_context(tc.tile_pool(name="work", bufs=1))

        # weight stream first (first qkv matmul needs wj0), then hidden
        # trickled per-k so the first qkv tile starts early
        wj_sb = {}

        def emit_wj(j):
            wj_sb[j] = work.tile([128, NK * 128], bf16, tag=f"wj{j % 6}",
                                 name=f"wj{j}", bufs=1)
            nc.sync.dma_start(wj_sb[j][:],
                              wqkvT[:, j * NK * 128:(j + 1) * NK * 128])

        for j in (0, 1):
            emit_wj(j)
        hid_sb = []
        for k in range(NK):
            t = work.tile([128, S_core], bf16, tag=f"hid{k}", name=f"hid{k}")
            hid_sb.append(t)
        for k in range(3):
            nc.sync.dma_start(hid_sb[k][:], hidden3[:, k, :])
        # small persistents after the latency-critical first loads
        nc.sync.dma_start(bias_sb[:], bias2d[:])
        nc.sync.dma_start(cos_sb[:], cosD[:])
        nc.sync.dma_start(sin2_sb[:], sin2D[:])
        for k in range(3, NK):
            nc.sync.dma_start(hid_sb[k][:], hidden3[:, k, :])

        def hid_ap(k, h0, h1):
            return hid_sb[k][:, h0:h1]
        # dense proj weights (loaded mid-pipeline, see head loop)
        wp_sb = []
        for kt in range(n_pk):
            wp_sb.append(work.tile([128, D], bf16, tag=f"wp{kt}",
                                   name=f"wp{kt}"))

        qkv_sb = {}     # j -> tile (ring of 8)
        rot_sb = {}     # (sec, h) -> tile (ring of 4)
        vaug_sb = {}    # h -> tile (ring of 2)
        packed = [work.tile([128, S_core], bf16, tag=f"pk{kt}",
                            name=f"pk{kt}") for kt in range(n_pk)]

        def emit_qkv_half(j, hi):
                if hi == 0:
                    qkv_sb[j] = work.tile([128, S_core], mm_dt,
                                          tag=f"qkv{j % 6}", name=f"qkvT{j}",
                                          bufs=1)
                h0, h1 = halves(0, S_core)[hi]
                hw = h1 - h0
                wj = wj_sb[j]
                ps = psum.tile([128, 512], f32, tag=f"mm{hi % 2}",
                               name="qkvp")
                for k in range(NK):
                    nc.tensor.matmul(
                        ps[:, :hw],
                        wj[:, k * 128:(k + 1) * 128],
                        hid_ap(k, h0, h1),
                        start=(k == 0), stop=(k == NK - 1))
                nc.scalar.activation(qkv_sb[j][:, h0:h1], ps[:, :hw],
                                     AF.Identity,
                                     bias=bias_sb[:, j:j + 1])

        def emit_qkv_j(j):
            for hi in range(len(halves(0, S_core))):
                emit_qkv_half(j, hi)

        def emit_wj_prefetch(h):
            for j in (2 * h, 2 * h + 1):
                if j < n_mtiles and j not in wj_sb:
                    emit_wj(j)

        def emit_rope_sec(h, sec):
            # stage x and swap(x) via SWDGE (Pool) DMA, then
            # rot = x*cos + swap(x)*sin2 on DVE (one mul on Pool)
            for sec in (sec,):
                lo_t, lo_r = pos[(sec, h, 0)]
                hi_t, hi_r = pos[(sec, h, 1)]
                assert hi_t == lo_t and hi_r == lo_r + BLK
                x = qkv_sb[lo_t]
                sa = work.tile([HD, S_core], mm_dt,
                               tag=f"sa{0 if sec == 'q' else 1}", name="sa",
                               bufs=2)
                sb = work.tile([HD, S_core], mm_dt,
                               tag=f"sb{0 if sec == 'q' else 1}", name="sb",
                               bufs=2)
                # sa via SP, sb via Pool: the two staging paths overlap, and
                # mulA (DVE) runs in parallel with mulB (Pool)
                nc.sync.dma_start(sa[0:HD, :], x[lo_r:lo_r + HD, :])
                nc.gpsimd.dma_start(sb[0:BLK, :], x[hi_r:hi_r + BLK, :])
                nc.gpsimd.dma_start(sb[BLK:HD, :], x[lo_r:lo_r + BLK, :])
                rot = work.tile([HD, S_core], mm_dt,
                                tag=f"rot_{sec}{h % 2}", name=f"rot_{sec}{h}",
                                bufs=1)
                rot_sb[(sec, h)] = rot
                nc.vector.tensor_mul(rot[0:HD, :], sa[0:HD, :],
                                     cos_sb[0:HD, :])
                nc.gpsimd.tensor_mul(sb[0:HD, :], sb[0:HD, :],
                                     sin2_sb[0:HD, :])
                nc.vector.tensor_add(rot[0:HD, :], rot[0:HD, :], sb[0:HD, :])

        def emit_rope(h):
            emit_rope_sec(h, "q")
            emit_rope_sec(h, "k")

        GRP = 4  # key tiles transposed per psum tile / copy

        def emit_vaug(h):
            va = work.tile([128, n_tt * VW], mm_dt, tag=f"vaug{h % 2}",
                           name=f"vaug{h}", bufs=1)
            vaug_sb[h] = va
            va3 = va.rearrange("p (t c) -> p t c", c=VW)
            vi3 = vinitD.rearrange("p (t c) -> p t c", c=1)
            nc.gpsimd.dma_start(va3[:, :, 0:1], vi3[:, :, :])
            gi = 0
            gidx = 0
            while gi < n_tt:
                ng = min(GRP, n_tt - gi)
                tp = psum.tile([128, 512], f32, tag=f"x{gidx % 2}", name="tp")
                for x in range(ng):
                    t0 = (gi + x) * 128
                    for half in (0, 1):
                        vt, vr = pos[("v", h, half)]
                        nc.tensor.transpose(
                            tp[:, x * HD + half * BLK:
                               x * HD + (half + 1) * BLK],
                            qkv_sb[vt][0:BLK, t0:t0 + 128].bitcast(f32),
                            ident[:BLK, :BLK])
                src = tp[:, :ng * HD].rearrange("p (t c) -> p t c", c=HD)
                if gidx % 2 == 0:
                    nc.vector.tensor_copy(va3[:, gi:gi + ng, 1:1 + HD],
                                          src[:, :, :])
                else:
                    nc.scalar.activation(va3[:, gi:gi + ng, 1:1 + HD],
                                         src[:, :, :], AF.Identity)
                gi += ng
                gidx += 1

        att_po = {}

        def emit_att_burst(h, ci, ti0, ti1):
            qT = rot_sb[("q", h)]
            kT = rot_sb[("k", h)]
            q0, q1 = chunks[ci]
            qs = q1 - q0
            if ti0 == 0:
                att_po[(h, ci)] = psum.tile([128, BA], f32,
                                            tag=f"po{ci % 2}", name="pv")
            po = att_po[(h, ci)]
            for ti in range(ti0, ti1):
                t0 = ti * 128
                ps = psum.tile([128, BA], f32, tag=f"st{ti % 2}", name="st")
                nc.tensor.matmul(ps[:, :qs], r_(kT[0:HD, t0:t0 + 128]),
                                 r_(qT[0:HD, q0:q1]),
                                 start=True, stop=True)
                pt = work.tile([128, BA], mm_dt, tag="pt", name="pt", bufs=3)
                nc.scalar.activation(pt[:, :qs], ps[:, :qs], AF.Exp)
                nc.tensor.matmul(
                    po[:VW, :qs],
                    r_(vaug_sb[h][:, ti * VW:(ti + 1) * VW]),
                    r_(pt[:, :qs]),
                    start=(ti == 0), stop=(ti == n_tt - 1))

        def emit_att_chunk(h, ci):
            emit_att_burst(h, ci, 0, n_tt)

        def emit_att_norm(h, ci):
            (q0, q1) = chunks[ci]
            qs = q1 - q0
            po = att_po.pop((h, ci))
            rc = work.tile([1, BA], f32, tag="rc", name="rc", bufs=2)
            nc.vector.reciprocal(rc[0:1, :qs], po[0:1, :qs])
            bc = work.tile([VW, BA], mm_dt, tag="bc", name="bc", bufs=2)
            nc.gpsimd.partition_broadcast(bc[0:VW, :qs],
                                          rc[0:1, :qs].bitcast(mm_dt))
            ast = work.tile([VW, BA], bf16, tag="ast", name="ast", bufs=3)
            nc.vector.tensor_mul(ast[0:VW, :qs], po[0:VW, :qs],
                                 bc[0:VW, :qs])
            for (t, r, n, off) in _pieces(HD * h, HD):
                nc.sync.dma_start(packed[t][r:r + n, q0:q1],
                                  ast[1 + off:1 + off + n, :qs])

        def emit_proj_chunk(ci):
            c0, c1 = chunks[ci]
            cs = c1 - c0
            for j in range(D // 128):
                ps = psum.tile([128, 512], f32, tag=f"mm{j % 2}", name="pj")
                for kt in range(n_pk):
                    nc.tensor.matmul(
                        ps[:, :cs],
                        wp_sb[kt][:, j * 128:(j + 1) * 128],
                        packed[kt][:, c0:c1],
                        start=(kt == 0), stop=(kt == n_pk - 1))
                ob = work.tile([128, BA], f32, tag="ob", name="ob", bufs=3)
                if j % 2 == 0:
                    nc.vector.tensor_copy(ob[:, :cs], ps[:, :cs])
                else:
                    nc.scalar.activation(ob[:, :cs], ps[:, :cs], AF.Identity)
                nc.scalar.dma_start(outT[j * 128:(j + 1) * 128, c0:c1],
                                    ob[:, :cs])

        # ---- pipeline: attention skewed one head behind qkv ----
        emit_qkv_j(0)
        emit_qkv_j(1)
        emit_wj_prefetch(1)
        emit_wj_prefetch(2)
        emit_rope(0)
        for h in range(1, n_h):
            emit_qkv_j(2 * h)
            emit_qkv_j(2 * h + 1)
            emit_wj_prefetch(h + 2)
            if h == 3:  # proj weights, needed only at the tail
                for kt in range(n_pk):
                    nc.sync.dma_start(wp_sb[kt][:], wp3[:, kt, :])
            emit_rope(h)
            emit_vaug(h - 1)
            for ci in range(len(chunks)):
                emit_att_chunk(h - 1, ci)
                emit_att_norm(h - 1, ci)
        # tail: head 7 chunk by chunk, hiding each norm+repack chain under
        # other PE work (the other chunk's attention / the projection)
        h7 = n_h - 1
        emit_vaug(h7)
        emit_att_chunk(h7, 0)
        emit_att_norm(h7, 0)
        emit_att_chunk(h7, 1)
        emit_proj_chunk(0)
        emit_att_norm(h7, 1)
        emit_proj_chunk(1)

    nc.compile()
    return nc


def _build_program(n_h, S_core, segs_local, resident_hidden):
    """Legacy SPMD program (mode C fallback). Same structure for every core."""
    import concourse.mybir as mybir
    import concourse.tile as tile
    from concourse import bacc
    from concourse.masks import make_identity
    from contextlib import ExitStack

    f32 = mybir.dt.float32
    mm_dt = getattr(mybir.dt, MM_DT_NAME)
    AF = mybir.ActivationFunctionType

    k_proj = n_h
    pos, n_mtiles = _pack_layout(n_h)
    dims_pad = n_mtiles * 128
    VW = 97  # v_aug slot width: 80 v dims + 16 zero pad + ones col at 96

    # global key-tile list: (seg_idx, t0, t1)
    t_tiles = []
    for si, (a, e) in enumerate(segs_local):
        t = a
        while t < e:
            t_tiles.append((si, t, min(t + 128, e)))
            t += 128
    n_tt = len(t_tiles)

    nc = bacc.Bacc("TRN2", target_bir_lowering=False, debug=False,
                   enable_asserts=False, num_devices=N_CORES)

    # host supplies hiddenT/wqkvT pre-tiled into 128-partition-major layout
    hiddenT = nc.dram_tensor("hiddenT", [128, NK * S_core], mm_dt,
                             kind="ExternalInput").ap()
    wqkvT = nc.dram_tensor("wqkvT", [128, NK * dims_pad], mm_dt,
                           kind="ExternalInput").ap()
    bias2d = nc.dram_tensor("bias2d", [128, n_mtiles], f32,
                            kind="ExternalInput").ap()
    # cosP/sin2P are host-packed [128, S]: rows 0:40 and 64:104 hold the
    # lo/hi rope coefficients, all other rows zero (zeroes the junk rows
    # of the rotated q/k so the K=104 score matmuls see exact zeros).
    cosP = nc.dram_tensor("cosP", [128, S_core], mm_dt,
                          kind="ExternalInput").ap()
    sin2P = nc.dram_tensor("sin2P", [128, S_core], mm_dt,
                           kind="ExternalInput").ap()
    wprojT = nc.dram_tensor("wprojT", [n_h * HD, D], mm_dt,
                            kind="ExternalInput").ap()
    # per-key-tile v_aug tail init: 16 zero pad cols + ones col (f32r memset
    # fails walrus codegen, so this comes in via DMA)
    vinit = nc.dram_tensor("vinit", [128, n_tt * (VW - HD)], mm_dt,
                           kind="ExternalInput").ap()
    outT = nc.dram_tensor("outT", [D, S_core], f32, kind="ExternalOutput").ap()

    def r_(ap):
        return ap.bitcast(mm_dt)

    BC = 1024  # psum tile width (2 banks); matmuls stream <=512
    big_chunks = [(c, min(c + BC, S_core)) for c in range(0, S_core, BC)]

    def halves(c0, c1):
        out = []
        q = c0
        while q < c1:
            out.append((q, min(q + 512, c1)))
            q = q + 512
        return out

    with tile.TileContext(nc) as tc, ExitStack() as ctx:
        persist = ctx.enter_context(tc.tile_pool(name="persist", bufs=1))
        ident = persist.tile([128, 128], f32, tag="ident", name="ident")
        make_identity(nc, ident[:])
        bias_sb = persist.tile([128, n_mtiles], f32, tag="bias", name="bias")
        nc.sync.dma_start(bias_sb[:], bias2d[:])

        psum_all_cm = tc.tile_pool(name="psum_all", bufs=1, space="PSUM")
        psum_all = psum_all_cm.__enter__()
        qkv_pool = ctx.enter_context(tc.tile_pool(name="big", bufs=1))
        qkv_sb = [qkv_pool.tile([128, S_core], mm_dt, tag=f"qkvT{j}",
                                name=f"qkvT{j}") for j in range(n_mtiles)]
        rot_cm = tc.tile_pool(name="rot", bufs=1)
        rv = rot_cm.__enter__()
        rot_sb = {}
        for h in range(n_h):
            for sec in ("q", "k"):
                rot_sb[(sec, h)] = rv.tile([128, S_core], mm_dt,
                                           tag=f"rot_{sec}{h}",
                                           name=f"rot_{sec}{h}")
        RC = 1024
        rope_cm = tc.tile_pool(name="rope_scr", bufs=2)
        rope_scr = rope_cm.__enter__()

        # ------------ phase 1: qkvT = Wpack @ hidden.T --------------
        with ExitStack() as p1:
            hidden3 = hiddenT.rearrange("p (k s) -> p k s", k=NK)
            w3 = wqkvT.rearrange("p (k m) -> p k m", k=NK)
            # k-outer streaming: two psum slots hold four j-streams
            # (columns 0:512 and 512:1024), hidden tiles are tiny
            w_pool = p1.enter_context(tc.tile_pool(name="wres", bufs=1))
            w_sb = [w_pool.tile([128, dims_pad], mm_dt, tag=f"w{k}",
                                name=f"w{k}") for k in range(NK)]
            for k in range(NK):
                nc.sync.dma_start(w_sb[k][:], w3[:, k, :])
            assert n_mtiles == 4
            hid_pool = p1.enter_context(tc.tile_pool(name="hidstream",
                                                     bufs=3))
            for (h0, h1) in halves(0, S_core):
                hw = h1 - h0
                ps01 = psum_all.tile([128, BC], f32, tag="t0", name="ps01")
                ps23 = psum_all.tile([128, BC], f32, tag="t1", name="ps23")
                pj_of = lambda j: (ps01 if j < 2 else ps23,
                                   (j % 2) * 512)
                for k in range(NK):
                    ht = hid_pool.tile([128, 512], mm_dt, tag="hidc",
                                       name="hidc")
                    nc.sync.dma_start(ht[:, :hw], hidden3[:, k, h0:h1])
                    for j in range(n_mtiles):
                        psj, co = pj_of(j)
                        nc.tensor.matmul(
                            psj[:, co:co + hw],
                            r_(w_sb[k][:, j * 128:(j + 1) * 128]),
                            r_(ht[:, :hw]),
                            start=(k == 0), stop=(k == NK - 1))
                for j in range(n_mtiles):
                    psj, co = pj_of(j)
                    nc.scalar.activation(qkv_sb[j][:, h0:h1],
                                         psj[:, co:co + hw], AF.Identity,
                                         bias=bias_sb[:, j:j + 1])

        psum_all_cm.__exit__(None, None, None)
        ps_att = ctx.enter_context(tc.tile_pool(name="ps_att", bufs=1,
                                                space="PSUM"))

        # ------------ phase 2: RoPE --------------------------------
        stg = {}
        for nm in ("sa0", "sa1", "sb0", "sb1"):
            stg[nm] = rope_scr.tile([128, RC], mm_dt, tag=nm, name=nm, bufs=1)
        pair_i = 0
        for ci, f0 in enumerate(range(0, S_core, RC)):
            f1 = min(f0 + RC, S_core)
            fs = f1 - f0
            cos_sb = rope_scr.tile([128, RC], mm_dt, tag="cos", name="cos",
                                   bufs=1)
            sin_sb = rope_scr.tile([128, RC], mm_dt, tag="sin", name="sin",
                                   bufs=1)
            nc.scalar.dma_start(cos_sb[:, :fs], cosP[:, f0:f1])
            nc.scalar.dma_start(sin_sb[:, :fs], sin2P[:, f0:f1])
            if ci == 0:
                for nm in stg:
                    nc.scalar.dma_start(stg[nm][BLK:64, :], cos_sb[BLK:64, :])
            for h in range(n_h):
                for sec in ("q", "k"):
                    lo_t, lo_r = pos[(sec, h, 0)]
                    hi_t, hi_r = pos[(sec, h, 1)]
                    assert hi_t == lo_t and hi_r == lo_r + BLK
                    x = qkv_sb[lo_t]
                    dst = rot_sb[(sec, h)]
                    stga = stg[f"sa{pair_i % 2}"]
                    stgb = stg[f"sb{pair_i % 2}"]
                    nc.scalar.dma_start(stga[0:BLK, :fs],
                                        x[lo_r:lo_r + BLK, f0:f1])
                    nc.scalar.dma_start(stga[64:64 + BLK, :fs],
                                        x[hi_r:hi_r + BLK, f0:f1])
                    nc.scalar.dma_start(stgb[0:BLK, :fs],
                                        x[hi_r:hi_r + BLK, f0:f1])
                    nc.scalar.dma_start(stgb[64:64 + BLK, :fs],
                                        x[lo_r:lo_r + BLK, f0:f1])
                    nc.vector.tensor_mul(dst[0:104, f0:f1], stga[0:104, :fs],
                                         cos_sb[0:104, :fs])
                    eng = nc.gpsimd if pair_i % 2 == 0 else nc.vector
                    eng.tensor_mul(stgb[0:104, :fs], stgb[0:104, :fs],
                                   sin_sb[0:104, :fs])
                    nc.vector.tensor_add(dst[0:104, f0:f1], dst[0:104, f0:f1],
                                         stgb[0:104, :fs])
                    pair_i += 1
        rope_cm.__exit__(None, None, None)

        vaug_cm = tc.tile_pool(name="vaug", bufs=1)
        vaug_pool = vaug_cm.__enter__()
        vaug_sb = [vaug_pool.tile([128, n_tt * VW], mm_dt, tag=f"vaug{h}",
                                  name=f"vaug{h}") for h in range(n_h)]
        vinit3 = vinit.rearrange("p (t c) -> p t c", c=VW - HD)
        for h in range(n_h):
            nc.sync.dma_start(
                vaug_sb[h].rearrange("p (t c) -> p t c", c=VW)[:, :, HD:VW],
                vinit3[:, :, :])
        GRP = 4  # key tiles transposed per psum tile / copy (1 psum bank)

        def emit_vaug(h):
            gi = 0
            while gi < n_tt:
                hi_g = min(gi + GRP, n_tt)
                if all(t_tiles[g][2] - t_tiles[g][1] == 128
                       for g in range(gi, hi_g)):
                    grp = list(range(gi, hi_g))
                else:
                    grp = [gi]
                ng = len(grp)
                tp = ps_att.tile([128, GRP * HD], f32, tag="tp", name="tp")
                for x, g in enumerate(grp):
                    si, t0, t1 = t_tiles[g]
                    sz = t1 - t0
                    for half in (0, 1):
                        vt, vr = pos[("v", h, half)]
                        nc.tensor.transpose(
                            tp[:sz, x * HD + half * BLK:
                               x * HD + (half + 1) * BLK],
                            qkv_sb[vt][0:BLK, t0:t1].bitcast(f32),
                            ident[:BLK, :BLK])
                sz0 = t_tiles[grp[0]][2] - t_tiles[grp[0]][1]
                dst = vaug_sb[h].rearrange("p (t c) -> p t c", c=VW)
                src_ap = tp.rearrange("p (t c) -> p t c", c=HD)
                if h % 2 == 0:
                    nc.vector.tensor_copy(dst[:sz0, grp[0]:grp[0] + ng, 0:HD],
                                          src_ap[:sz0, 0:ng, :])
                else:
                    nc.scalar.activation(dst[:sz0, grp[0]:grp[0] + ng, 0:HD],
                                         src_ap[:sz0, 0:ng, :], AF.Identity)
                gi += ng

        # ------------ phase 4: attention ----------------------------
        attn_sb = [qkv_pool.tile([128, S_core], mm_dt, tag=f"qkvT{h}",
                                 name=f"attnT{h}") for h in range(n_h)]

        seg_ttiles = {}
        for ti, (si, t0, t1) in enumerate(t_tiles):
            seg_ttiles.setdefault(si, []).append((ti, t0, t1))

        BA = 512  # attention query-chunk width (1-bank psum slots)
        with ExitStack() as p4:
            pt_pool = p4.enter_context(tc.tile_pool(name="pt", bufs=3))
            nrm_pool = p4.enter_context(tc.tile_pool(name="nrm", bufs=2))
            unit_box = [0]

            def emit_attention(h, si, a, e):
                qT = rot_sb[("q", h)]
                kT = rot_sb[("k", h)]
                q = a
                while q < e:
                    q0, q1 = q, min(q + BA, e)
                    qs = q1 - q0
                    po = ps_att.tile([128, BA], f32,
                                     tag=f"po{unit_box[0] % 2}", name="pv")
                    tts = seg_ttiles[si]
                    for idx, (ti, t0, t1) in enumerate(tts):
                        sz = t1 - t0
                        ps = ps_att.tile([128, BA], f32, tag=f"st{idx % 2}",
                                         name="st")
                        nc.tensor.matmul(ps[:sz, :qs], r_(kT[0:104, t0:t1]),
                                         r_(qT[0:104, q0:q1]),
                                         start=True, stop=True)
                        pt = pt_pool.tile([128, BA], mm_dt, tag="pt", name="pt")
                        nc.scalar.activation(pt[:sz, :qs], ps[:sz, :qs], AF.Exp)
                        nc.tensor.matmul(
                            po[:VW, :qs],
                            r_(vaug_sb[h][:sz, ti * VW:(ti + 1) * VW]),
                            r_(pt[:sz, :qs]),
                            start=(idx == 0), stop=(idx == len(tts) - 1))
                    # partition_broadcast ucode reads physical partition 0,
                    # so shift the denominator row 96 -> 0 via DMA
                    rc = nrm_pool.tile([128, BA], f32, tag="rc", name="rc")
                    nc.vector.tensor_copy(rc[96:97, :qs], po[96:97, :qs])
                    nc.sync.dma_start(rc[0:1, :qs], rc[96:97, :qs])
                    nc.vector.reciprocal(rc[0:1, :qs], rc[0:1, :qs])
                    bc = nrm_pool.tile([128, BA], mm_dt, tag="bc", name="bc")
                    nc.gpsimd.partition_broadcast(
                        bc[0:HD, :qs], rc[0:1, :qs].bitcast(mm_dt))
                    nc.vector.tensor_mul(attn_sb[h][0:HD, q0:q1],
                                         po[0:HD, :qs], bc[0:HD, :qs])
                    unit_box[0] += 1
                    q = q1

            for h in range(n_h):
                emit_vaug(h)
            for si, (a, e) in enumerate(segs_local):
                for h in range(n_h):
                    emit_attention(h, si, a, e)

        vaug_cm.__exit__(None, None, None)
        rot_cm.__exit__(None, None, None)

        # ------------ phase 5: projection partial -------------------
        with ExitStack() as p5:
            wp_pool = p5.enter_context(tc.tile_pool(name="wp", bufs=1))
            wp_sb = []
            for kt in range(k_proj):
                t = wp_pool.tile([HD, D], mm_dt, tag=f"wp{kt}", name=f"wp{kt}")
                nc.sync.dma_start(t[:], wprojT[kt * HD:(kt + 1) * HD, :])
                wp_sb.append(t)
            out_pool = p5.enter_context(tc.tile_pool(name="outsb", bufs=3))
            for (c0, c1) in big_chunks:
                cs = c1 - c0
                for j in range(D // 128):
                    ob = out_pool.tile([128, BC], f32, tag="ob", name="ob")
                    for (h0, h1) in halves(c0, c1):
                        ps = ps_att.tile([128, 512], f32, tag=f"st{j % 2}",
                                         name="pj")
                        for kt in range(k_proj):
                            nc.tensor.matmul(
                                ps[:, :h1 - h0],
                                r_(wp_sb[kt][:, j * 128:(j + 1) * 128]),
                                r_(attn_sb[kt][0:HD, h0:h1]),
                                start=(kt == 0), stop=(kt == k_proj - 1))
                        if j % 2 == 0:
                            nc.vector.tensor_copy(ob[:, h0 - c0:h1 - c0],
                                                  ps[:, :h1 - h0])
                        else:
                            nc.scalar.activation(ob[:, h0 - c0:h1 - c0],
                                                 ps[:, :h1 - h0], AF.Identity)
                    nc.sync.dma_start(outT[j * 128:(j + 1) * 128, c0:c1],
                                      ob[:, :cs])

    nc.compile()
    return nc


def _pack_w(Wqkv, bqkv, heads, n_h, jmajor=False):
    """Per-core packed qkv weights (q rows pre-scaled).

    Returns wqkvT_tiled [128, NK*dims_pad] (k-major blocks of [128,
    dims_pad], or j-major [128, n_mtiles*NK*128] when jmajor) and bias2d
    [128, n_mtiles]."""
    pos, n_mtiles = _pack_layout(n_h)
    dims_pad = n_mtiles * 128
    W = np.zeros((dims_pad, D), np.float32)
    b = np.zeros((dims_pad,), np.float32)
    sec_off = {"q": 0, "k": D, "v": 2 * D}
    for i, h in enumerate(heads):
        for sec in ("q", "k", "v"):
            for half in (0, 1):
                t, r = pos[(sec, i, half)]
                src = sec_off[sec] + h * HD + half * BLK
                w = Wqkv[src:src + BLK, :]
                bb = bqkv[src:src + BLK]
                if sec == "q":
                    w = w * SCALE
                    bb = bb * SCALE
                W[t * 128 + r:t * 128 + r + BLK] = w
                b[t * 128 + r:t * 128 + r + BLK] = bb
    WT = np.ascontiguousarray(W.T)  # [D = NK*128, dims_pad = n_mtiles*128]
    if jmajor:
        w_tiled = np.ascontiguousarray(
            WT.reshape(NK, 128, n_mtiles, 128).transpose(1, 2, 0, 3)
            .reshape(128, n_mtiles * NK * 128))
    else:
        w_tiled = _tile_rows(WT)
    bias2d = np.ascontiguousarray(b.reshape(n_mtiles, 128).T)
    return w_tiled, bias2d


def _tile_rows(x):
    """[R, C] with R = nk*128 -> [128, nk*C] k-major tiling."""
    R, C = x.shape
    nk = R // 128
    return np.ascontiguousarray(
        x.reshape(nk, 128, C).transpose(1, 0, 2).reshape(128, nk * C))


def _pack_wproj(Wproj, heads):
    """Rows of Wproj.T for this core's head dims, stacked per head."""
    W = np.zeros((len(heads) * HD, Wproj.shape[0]), np.float32)
    for i, h in enumerate(heads):
        W[i * HD:(i + 1) * HD] = Wproj[:, h * HD:(h + 1) * HD].T
    return W


def _pack_cos_sin(cos, sin):
    """cosP/sin2P [128, S]: lo coeffs at rows 0:40, hi at 64:104, rest 0.

    sin2P row signs match rot = x*cosP + swap(x)*sin2P: lo rows hold
    -sin_lo (they multiply x_hi), hi rows hold +sin_hi (they multiply x_lo).
    """
    S = cos.shape[0]
    cosP = np.zeros((128, S), np.float32)
    sinP = np.zeros((128, S), np.float32)
    cosP[0:BLK] = cos.T[0:BLK]
    cosP[64:64 + BLK] = cos.T[BLK:HD]
    sinP[0:BLK] = -sin.T[0:BLK]
    sinP[64:64 + BLK] = sin.T[BLK:HD]
    return cosP, sinP


def _pack_cos_sin_dense(cos, sin):
    """Dense [80, S] rope coefficients for mode A.

    sin2D row signs match rot = x*cos + swap(x)*sin2D: rows 0:40 hold
    -sin_lo (they multiply x_hi), rows 40:80 hold +sin_hi (x_lo)."""
    cosT = np.ascontiguousarray(cos.T.astype(np.float32))
    sinT = sin.T.astype(np.float32)
    sin2 = np.concatenate([-sinT[0:BLK], sinT[BLK:HD]], axis=0)
    return cosT, np.ascontiguousarray(sin2)


_CACHE = {}


def kernel(hidden_states, cos, sin, Wqkv, bqkv, Wproj, bproj, cu_seqlens):
    sys.path.insert(0, "/opt/trn_rl_repo")
    from concourse import bass_utils

    hidden_states = np.asarray(hidden_states, np.float32)
    cos = np.asarray(cos, np.float32)
    sin = np.asarray(sin, np.float32)
    Wqkv = np.asarray(Wqkv, np.float32)
    bqkv = np.asarray(bqkv, np.float32)
    Wproj = np.asarray(Wproj, np.float32)
    bproj = np.asarray(bproj, np.float32)

    S, D_ = hidden_states.shape
    assert D_ == D
    segs = _segments(cu_seqlens, S)
    uniform = (S % 4 == 0) and segs == [(i * S // 4, (i + 1) * S // 4)
                                        for i in range(4)]

    hiddenT = np.ascontiguousarray(hidden_states.T)

    if uniform:
        # mode A: 2 head-groups x 4 segments, pipelined program
        n_h, S_core = H // 2, S // 4
        key = ("A", S)
        if key not in _CACHE:
            _CACHE[key] = _build_program_a(n_h, S_core)
        nc = _CACHE[key]
        import ml_dtypes
        bf = ml_dtypes.bfloat16
        cosD, sin2D = _pack_cos_sin_dense(cos, sin)
        n_tt = S_core // 128
        vinitD = np.ones((128, n_tt), np.float32)
        in_maps = []
        meta = []
        for g in range(2):
            heads = list(range(g * n_h, (g + 1) * n_h))
            wt, b2 = _pack_w(Wqkv, bqkv, heads, n_h, jmajor=True)
            wt = wt.astype(bf)
            wprojP = _tile_rows(_pack_wproj(Wproj, heads)).astype(bf)
            for s in range(4):
                sl = slice(s * S_core, (s + 1) * S_core)
                in_maps.append({
                    "hiddenT": _tile_rows(hiddenT[:, sl]).astype(bf),
                    "wqkvT": wt,
                    "bias2d": b2,
                    "cosD": np.ascontiguousarray(cosD[:, sl]),
                    "sin2D": np.ascontiguousarray(sin2D[:, sl]),
                    "wprojP": wprojP,
                    "vinitD": vinitD,
                })
                meta.append((g, s))
        res = bass_utils.run_bass_kernel_spmd(nc, in_maps,
                                              core_ids=list(range(N_CORES)))
        out = np.zeros((D, S), np.float32)
        for c, (g, s) in enumerate(meta):
            out[:, s * S_core:(s + 1) * S_core] += res.results[c]["outT"]
    else:
        # mode C: 8-way head parallel, full sequence per core
        n_h, S_core = H // N_CORES, S
        key = ("C", S, tuple(np.asarray(cu_seqlens).tolist()))
        if key not in _CACHE:
            _CACHE[key] = _build_program(n_h, S_core, segs,
                                         resident_hidden=False)
        nc = _CACHE[key]
        cosP, sin2P = _pack_cos_sin(cos, sin)

        def _vinit(segs_local):
            n_tt = sum(-(-(e - a) // 128) for a, e in segs_local)
            v = np.zeros((128, n_tt, 17), np.float32)
            v[:, :, 16] = 1.0
            return np.ascontiguousarray(v.reshape(128, n_tt * 17))

        vinit = _vinit(segs)
        hid_tiled = _tile_rows(hiddenT)
        in_maps = []
        for c in range(N_CORES):
            heads = list(range(c * n_h, (c + 1) * n_h))
            wt, b2 = _pack_w(Wqkv, bqkv, heads, n_h)
            in_maps.append({
                "hiddenT": hid_tiled,
                "wqkvT": wt,
                "bias2d": b2,
                "cosP": cosP,
                "sin2P": sin2P,
                "wprojT": _pack_wproj(Wproj, heads),
                "vinit": vinit,
            })
        res = bass_utils.run_bass_kernel_spmd(nc, in_maps,
                                              core_ids=list(range(N_CORES)))
        out = np.zeros((D, S), np.float32)
        for c in range(N_CORES):
            out += res.results[c]["outT"]

    return np.ascontiguousarray(out.T) + bproj[None, :]


# revision 49
# speedup vs baseline: 1.0299x; 1.0118x over previous
"""Trainium2 Bass kernel for Ernie4.5-VL vision attention (ragged segments).

Contract: kernel(**inputs) takes the FULL unsharded inputs (keyed as in
setup_inputs()) and returns the FULL [S, D] float32 output.

Strategy
--------
All matmuls run on the PE array in float32r (full-rate fp32, ~1.5e-4 rel
err); everything else is fp32. Attention is computed per segment
(block-diagonal, no masks) in a flash-like streaming form that only ever
materializes transposed score tiles.

Mode A (uniform 4x1024 segments, the common case): 2 head-groups x 4
segments across 8 cores; each core runs a per-head software pipeline with
skew 1:

  iter h: qkv j-tiles (2h, 2h+1)  [PE]  ->  rope head h  [Pool/DVE DMA+mul]
          vaug + attention head h-1      [PE transposes, ACT exp, PE PV]
  tail:   attention head 7, then dense projection (5x128 contraction
          tiles assembled by DMA-repacking the per-head attention output)

Engine budget per head: PE ~15.6us (20 qkv mm + 16 transposes + 32 attn
mm), ACT ~10.4us (4 evac + 16 exp), DVE ~7us (rope muls, recip,
normalize), Pool ~11us (rope staging SWDGE DMAs, one rope mul, vaug
copies, partition broadcasts). PE is the critical engine; everything
else hides behind it.

Mode C (any other cu_seqlens): legacy 8-way head-parallel program, every
core sees all segments.

Host does only O(S*D) glue: input transposes/packing, summing the 2 (or
8) per-token partial projections, and the bias adds.
"""

import os
import sys

import numpy as np

H = 16
HD = 80
BLK = 40  # rotate_half half-width
SCALE = HD ** -0.5
N_CORES = 8
D = 1280
NK = D // 128  # contraction tiles for the qkv matmul
ATTN_STRIDE = 96  # head row pitch in the packed attention output (legacy)
MM_DT_NAME = os.environ.get("KERNEL_MM_DT", "float32r")  # or "float32"
KERNEL_DEBUG = bool(int(os.environ.get("KERNEL_DEBUG", "0")))


def _segments(cu_seqlens, S):
    """Intervals matching reference's searchsorted(cu[1:], i, 'right')."""
    b = np.clip(np.sort(np.asarray(cu_seqlens, dtype=np.int64)[1:5]), 0, S)
    bounds = [0] + list(b) + [S]
    segs = []
    for a, e in zip(bounds[:-1], bounds[1:]):
        if e > a:
            segs.append((int(a), int(e)))
    return segs


def _pack_layout(n_h):
    """Pack per-core qkv dims as 40-row blocks, 3 per 128-row tile (8 pad).

    Each tile holds one v-block at row 0 (PE transpose operands must start
    at a 32-aligned partition) and two q/k blocks at rows 40 and 80.
    Returns pos[(sec, h, half)] = (tile, row) and the number of tiles.
    """
    ntiles = 2 * n_h
    pos = {}
    for h in range(n_h):
        pos[("v", h, 0)] = (2 * h, 0)
        pos[("v", h, 1)] = (2 * h + 1, 0)
        pos[("q", h, 0)] = (2 * h, BLK)
        pos[("q", h, 1)] = (2 * h, 2 * BLK)
        pos[("k", h, 0)] = (2 * h + 1, BLK)
        pos[("k", h, 1)] = (2 * h + 1, 2 * BLK)
    return pos, ntiles


def _pieces(start, length, tile_rows=128):
    """Split global row range [start, start+length) into per-tile pieces."""
    out = []
    off = 0
    while off < length:
        g = start + off
        t, r = g // tile_rows, g % tile_rows
        n = min(tile_rows - r, length - off)
        out.append((t, r, n, off))
        off += n
    return out


def _proj_k_tiles(n_h):
    rows = ATTN_STRIDE * n_h
    kt = [128] * (rows // 128)
    if rows % 128:
        kt.append(rows % 128)
    return kt


def _build_program_a(n_h, S_core):
    """Pipelined SPMD program for mode A (single segment per core).

    Engine-AP partition rules on TRN2 (walrus birverifier): compute-engine
    accesses must start at a 32-aligned partition and must not cross a
    64-boundary unless they start on one; cross-partition data movement
    must go through DMA. The layout choices below all follow from this.
    """
    import concourse.mybir as mybir
    import concourse.tile as tile
    from concourse import bacc
    from concourse.masks import make_identity
    from contextlib import ExitStack

    f32 = mybir.dt.float32
    bf16 = mybir.dt.bfloat16
    mm_dt = getattr(mybir.dt, MM_DT_NAME)
    AF = mybir.ActivationFunctionType

    pos, n_mtiles = _pack_layout(n_h)
    dims_pad = n_mtiles * 128
    VW = 81  # v_aug slot: ones col at 0 (-> denominator on psum row 0), v at 1:81
    n_tt = S_core // 128
    assert S_core % 128 == 0
    BA = 512
    chunks = [(c, min(c + BA, S_core)) for c in range(0, S_core, BA)]
    n_pk = (n_h * HD) // 128  # dense proj contraction tiles
    assert (n_h * HD) % 128 == 0

    nc = bacc.Bacc("TRN2", target_bir_lowering=False, debug=False,
                   enable_asserts=False, num_devices=N_CORES)

    # the two big input streams come in as bf16 (halves HBM traffic; rel
    # err contribution ~4e-3, well under the 2e-2 gate); attention math
    # stays f32r end to end. wqkvT is j-major: tile j's columns contiguous.
    hiddenT = nc.dram_tensor("hiddenT", [128, NK * S_core], bf16,
                             kind="ExternalInput").ap()
    wqkvT = nc.dram_tensor("wqkvT", [128, n_mtiles * NK * 128], bf16,
                           kind="ExternalInput").ap()
    bias2d = nc.dram_tensor("bias2d", [128, n_mtiles], f32,
                            kind="ExternalInput").ap()
    cosD = nc.dram_tensor("cosD", [HD, S_core], mm_dt,
                          kind="ExternalInput").ap()
    # sin2D rows 0:40 hold -sin_lo (they multiply x_hi), rows 40:80 hold
    # +sin_hi (they multiply x_lo); staging swaps the halves of x.
    sin2D = nc.dram_tensor("sin2D", [HD, S_core], mm_dt,
                           kind="ExternalInput").ap()
    wprojP = nc.dram_tensor("wprojP", [128, n_pk * D], bf16,
                            kind="ExternalInput").ap()
    vinitD = nc.dram_tensor("vinitD", [128, n_tt], mm_dt,
                            kind="ExternalInput").ap()
    outT = nc.dram_tensor("outT", [D, S_core], f32, kind="ExternalOutput").ap()

    def r_(ap):
        return ap.bitcast(mm_dt)

    def halves(c0, c1):
        out = []
        q = c0
        while q < c1:
            out.append((q, min(q + 512, c1)))
            q = q + 512
        return out

    hidden3 = hiddenT.rearrange("p (k s) -> p k s", k=NK)
    wp3 = wprojP.rearrange("p (k m) -> p k m", k=n_pk)

    with tile.TileContext(nc) as tc, ExitStack() as ctx:
        persist = ctx.enter_context(tc.tile_pool(name="persist", bufs=1))
        ident = persist.tile([128, 128], f32, tag="ident", name="ident")
        make_identity(nc, ident[:])
        bias_sb = persist.tile([128, n_mtiles], f32, tag="bias", name="bias")
        cos_sb = persist.tile([HD, S_core], mm_dt, tag="cos", name="cos")
        sin2_sb = persist.tile([HD, S_core], mm_dt, tag="sin2", name="sin2")

        psum = ctx.enter_context(tc.tile_pool(name="psum", bufs=1,
                                              space="PSUM"))
        work = ctx.enter_context(tc.tile_pool(name="work", bufs=1))

        # weight stream first (first qkv matmul needs wj0), then hidden
        # trickled per-k so the first qkv tile starts early
        wj_sb = {}

        def emit_wj(j):
            wj_sb[j] = work.tile([128, NK * 128], bf16, tag=f"wj{j % 6}",
                                 name=f"wj{j}", bufs=1)
            nc.sync.dma_start(wj_sb[j][:],
                              wqkvT[:, j * NK * 128:(j + 1) * NK * 128])

        for j in (0, 1):
            emit_wj(j)
        hid_sb = []
        for k in range(NK):
            t = work.tile([128, S_core], bf16, tag=f"hid{k}", name=f"hid{k}")
            hid_sb.append(t)
        for k in range(3):
            nc.sync.dma_start(hid_sb[k][:], hidden3[:, k, :])
        # small persistents after the latency-critical first loads
        nc.sync.dma_start(bias_sb[:], bias2d[:])
        nc.sync.dma_start(cos_sb[:], cosD[:])
        nc.sync.dma_start(sin2_sb[:], sin2D[:])
        for k in range(3, NK):
            nc.sync.dma_start(hid_sb[k][:], hidden3[:, k, :])

        def hid_ap(k, h0, h1):
            return hid_sb[k][:, h0:h1]
        # dense proj weights (loaded mid-pipeline, see head loop)
        wp_sb = []
        for kt in range(n_pk):
            wp_sb.append(work.tile([128, D], bf16, tag=f"wp{kt}",
                                   name=f"wp{kt}"))

        qkv_sb = {}     # j -> tile (ring of 8)
        rot_sb = {}     # (sec, h) -> tile (ring of 4)
        vaug_sb = {}    # h -> tile (ring of 2)
        packed = [work.tile([128, S_core], bf16, tag=f"pk{kt}",
                            name=f"pk{kt}") for kt in range(n_pk)]

        def emit_qkv_half(j, hi):
                if hi == 0:
                    qkv_sb[j] = work.tile([128, S_core], mm_dt,
                                          tag=f"qkv{j % 6}", name=f"qkvT{j}",
                                          bufs=1)
                h0, h1 = halves(0, S_core)[hi]
                hw = h1 - h0
                wj = wj_sb[j]
                ps = psum.tile([128, 512], f32, tag=f"mm{hi % 2}",
                               name="qkvp")
                for k in range(NK):
                    nc.tensor.matmul(
                        ps[:, :hw],
                        wj[:, k * 128:(k + 1) * 128],
                        hid_ap(k, h0, h1),
                        start=(k == 0), stop=(k == NK - 1))
                nc.scalar.activation(qkv_sb[j][:, h0:h1], ps[:, :hw],
                                     AF.Identity,
                                     bias=bias_sb[:, j:j + 1])

        def emit_qkv_j(j):
            for hi in range(len(halves(0, S_core))):
                emit_qkv_half(j, hi)

        def emit_wj_prefetch(h):
            for j in (2 * h, 2 * h + 1):
                if j < n_mtiles and j not in wj_sb:
                    emit_wj(j)

        def emit_rope_sec(h, sec):
            # stage x and swap(x) via SWDGE (Pool) DMA, then
            # rot = x*cos + swap(x)*sin2 on DVE (one mul on Pool)
            for sec in (sec,):
                lo_t, lo_r = pos[(sec, h, 0)]
                hi_t, hi_r = pos[(sec, h, 1)]
                assert hi_t == lo_t and hi_r == lo_r + BLK
                x = qkv_sb[lo_t]
                sa = work.tile([HD, S_core], mm_dt,
                               tag=f"sa{0 if sec == 'q' else 1}", name="sa",
                               bufs=2)
                sb = work.tile([HD, S_core], mm_dt,
                               tag=f"sb{0 if sec == 'q' else 1}", name="sb",
                               bufs=2)
                nc.gpsimd.dma_start(sa[0:HD, :], x[lo_r:lo_r + HD, :])
                nc.gpsimd.dma_start(sb[0:BLK, :], x[hi_r:hi_r + BLK, :])
                nc.gpsimd.dma_start(sb[BLK:HD, :], x[lo_r:lo_r + BLK, :])
                rot = work.tile([HD, S_core], mm_dt,
                                tag=f"rot_{sec}{h % 2}", name=f"rot_{sec}{h}",
                                bufs=1)
                rot_sb[(sec, h)] = rot
                nc.vector.tensor_mul(rot[0:HD, :], sa[0:HD, :],
                                     cos_sb[0:HD, :])
                eng = nc.gpsimd if sec == "q" else nc.vector
                eng.tensor_mul(sb[0:HD, :], sb[0:HD, :], sin2_sb[0:HD, :])
                nc.vector.tensor_add(rot[0:HD, :], rot[0:HD, :], sb[0:HD, :])

        def emit_rope(h):
            emit_rope_sec(h, "q")
            emit_rope_sec(h, "k")

        GRP = 4  # key tiles transposed per psum tile / copy

        def emit_vaug(h):
            va = work.tile([128, n_tt * VW], mm_dt, tag=f"vaug{h % 2}",
                           name=f"vaug{h}", bufs=1)
            vaug_sb[h] = va
            va3 = va.rearrange("p (t c) -> p t c", c=VW)
            vi3 = vinitD.rearrange("p (t c) -> p t c", c=1)
            nc.gpsimd.dma_start(va3[:, :, 0:1], vi3[:, :, :])
            gi = 0
            gidx = 0
            while gi < n_tt:
                ng = min(GRP, n_tt - gi)
                tp = psum.tile([128, 512], f32, tag=f"x{gidx % 2}", name="tp")
                for x in range(ng):
                    t0 = (gi + x) * 128
                    for half in (0, 1):
                        vt, vr = pos[("v", h, half)]
                        nc.tensor.transpose(
                            tp[:, x * HD + half * BLK:
                               x * HD + (half + 1) * BLK],
                            qkv_sb[vt][0:BLK, t0:t0 + 128].bitcast(f32),
                            ident[:BLK, :BLK])
                src = tp[:, :ng * HD].rearrange("p (t c) -> p t c", c=HD)
                if gidx % 2 == 0:
                    nc.vector.tensor_copy(va3[:, gi:gi + ng, 1:1 + HD],
                                          src[:, :, :])
                else:
                    nc.scalar.activation(va3[:, gi:gi + ng, 1:1 + HD],
                                         src[:, :, :], AF.Identity)
                gi += ng
                gidx += 1

        att_po = {}

        def emit_att_burst(h, ci, ti0, ti1):
            qT = rot_sb[("q", h)]
            kT = rot_sb[("k", h)]
            q0, q1 = chunks[ci]
            qs = q1 - q0
            if ti0 == 0:
                att_po[(h, ci)] = psum.tile([128, BA], f32,
                                            tag=f"po{ci % 2}", name="pv")
            po = att_po[(h, ci)]
            for ti in range(ti0, ti1):
                t0 = ti * 128
                ps = psum.tile([128, BA], f32, tag=f"st{ti % 2}", name="st")
                nc.tensor.matmul(ps[:, :qs], r_(kT[0:HD, t0:t0 + 128]),
                                 r_(qT[0:HD, q0:q1]),
                                 start=True, stop=True)
                pt = work.tile([128, BA], mm_dt, tag="pt", name="pt", bufs=3)
                nc.scalar.activation(pt[:, :qs], ps[:, :qs], AF.Exp)
                nc.tensor.matmul(
                    po[:VW, :qs],
                    r_(vaug_sb[h][:, ti * VW:(ti + 1) * VW]),
                    r_(pt[:, :qs]),
                    start=(ti == 0), stop=(ti == n_tt - 1))

        def emit_att_chunk(h, ci):
            emit_att_burst(h, ci, 0, n_tt)

        def emit_att_norm(h, ci):
            (q0, q1) = chunks[ci]
            qs = q1 - q0
            po = att_po.pop((h, ci))
            rc = work.tile([1, BA], f32, tag="rc", name="rc", bufs=2)
            nc.vector.reciprocal(rc[0:1, :qs], po[0:1, :qs])
            bc = work.tile([VW, BA], mm_dt, tag="bc", name="bc", bufs=2)
            nc.gpsimd.partition_broadcast(bc[0:VW, :qs],
                                          rc[0:1, :qs].bitcast(mm_dt))
            ast = work.tile([VW, BA], bf16, tag="ast", name="ast", bufs=3)
            nc.vector.tensor_mul(ast[0:VW, :qs], po[0:VW, :qs],
                                 bc[0:VW, :qs])
            for (t, r, n, off) in _pieces(HD * h, HD):
                nc.sync.dma_start(packed[t][r:r + n, q0:q1],
                                  ast[1 + off:1 + off + n, :qs])

        def emit_proj_chunk(ci):
            c0, c1 = chunks[ci]
            cs = c1 - c0
            for j in range(D // 128):
                ps = psum.tile([128, 512], f32, tag=f"mm{j % 2}", name="pj")
                for kt in range(n_pk):
                    nc.tensor.matmul(
                        ps[:, :cs],
                        wp_sb[kt][:, j * 128:(j + 1) * 128],
                        packed[kt][:, c0:c1],
                        start=(kt == 0), stop=(kt == n_pk - 1))
                ob = work.tile([128, BA], f32, tag="ob", name="ob", bufs=3)
                if j % 2 == 0:
                    nc.vector.tensor_copy(ob[:, :cs], ps[:, :cs])
                else:
                    nc.scalar.activation(ob[:, :cs], ps[:, :cs], AF.Identity)
                nc.scalar.dma_start(outT[j * 128:(j + 1) * 128, c0:c1],
                                    ob[:, :cs])

        # ---- pipeline: attention skewed one head behind qkv ----
        emit_qkv_j(0)
        emit_qkv_j(1)
        emit_wj_prefetch(1)
        emit_wj_prefetch(2)
        emit_rope(0)
        for h in range(1, n_h):
            emit_qkv_j(2 * h)
            emit_qkv_j(2 * h + 1)
            emit_wj_prefetch(h + 2)
            if h == 3:  # proj weights, needed only at the tail
                for kt in range(n_pk):
                    nc.sync.dma_start(wp_sb[kt][:], wp3[:, kt, :])
            emit_rope(h)
            emit_vaug(h - 1)
            for ci in range(len(chunks)):
                emit_att_chunk(h - 1, ci)
                emit_att_norm(h - 1, ci)
        # tail: head 7 chunk by chunk, hiding each norm+repack chain under
        # other PE work (the other chunk's attention / the projection)
        h7 = n_h - 1
        emit_vaug(h7)
        emit_att_chunk(h7, 0)
        emit_att_norm(h7, 0)
        emit_att_chunk(h7, 1)
        emit_proj_chunk(0)
        emit_att_norm(h7, 1)
        emit_proj_chunk(1)

    nc.compile()
    return nc


def _build_program(n_h, S_core, segs_local, resident_hidden):
    """Legacy SPMD program (mode C fallback). Same structure for every core."""
    import concourse.mybir as mybir
    import concourse.tile as tile
    from concourse import bacc
    from concourse.masks import make_identity
    from contextlib import ExitStack

    f32 = mybir.dt.float32
    mm_dt = getattr(mybir.dt, MM_DT_NAME)
    AF = mybir.ActivationFunctionType

    k_proj = n_h
    pos, n_mtiles = _pack_layout(n_h)
    dims_pad = n_mtiles * 128
    VW = 97  # v_aug slot width: 80 v dims + 16 zero pad + ones col at 96

    # global key-tile list: (seg_idx, t0, t1)
    t_tiles = []
    for si, (a, e) in enumerate(segs_local):
        t = a
        while t < e:
            t_tiles.append((si, t, min(t + 128, e)))
            t += 128
    n_tt = len(t_tiles)

    nc = bacc.Bacc("TRN2", target_bir_lowering=False, debug=False,
                   enable_asserts=False, num_devices=N_CORES)

    # host supplies hiddenT/wqkvT pre-tiled into 128-partition-major layout
    hiddenT = nc.dram_tensor("hiddenT", [128, NK * S_core], mm_dt,
                             kind="ExternalInput").ap()
    wqkvT = nc.dram_tensor("wqkvT", [128, NK * dims_pad], mm_dt,
                           kind="ExternalInput").ap()
    bias2d = nc.dram_tensor("bias2d", [128, n_mtiles], f32,
                            kind="ExternalInput").ap()
    # cosP/sin2P are host-packed [128, S]: rows 0:40 and 64:104 hold the
    # lo/hi rope coefficients, all other rows zero (zeroes the junk rows
    # of the rotated q/k so the K=104 score matmuls see exact zeros).
    cosP = nc.dram_tensor("cosP", [128, S_core], mm_dt,
                          kind="ExternalInput").ap()
    sin2P = nc.dram_tensor("sin2P", [128, S_core], mm_dt,
                           kind="ExternalInput").ap()
    wprojT = nc.dram_tensor("wprojT", [n_h * HD, D], mm_dt,
                            kind="ExternalInput").ap()
    # per-key-tile v_aug tail init: 16 zero pad cols + ones col (f32r memset
    # fails walrus codegen, so this comes in via DMA)
    vinit = nc.dram_tensor("vinit", [128, n_tt * (VW - HD)], mm_dt,
                           kind="ExternalInput").ap()
    outT = nc.dram_tensor("outT", [D, S_core], f32, kind="ExternalOutput").ap()

    def r_(ap):
        return ap.bitcast(mm_dt)

    BC = 1024  # psum tile width (2 banks); matmuls stream <=512
    big_chunks = [(c, min(c + BC, S_core)) for c in range(0, S_core, BC)]

    def halves(c0, c1):
        out = []
        q = c0
        while q < c1:
            out.append((q, min(q + 512, c1)))
            q = q + 512
        return out

    with tile.TileContext(nc) as tc, ExitStack() as ctx:
        persist = ctx.enter_context(tc.tile_pool(name="persist", bufs=1))
        ident = persist.tile([128, 128], f32, tag="ident", name="ident")
        make_identity(nc, ident[:])
        bias_sb = persist.tile([128, n_mtiles], f32, tag="bias", name="bias")
        nc.sync.dma_start(bias_sb[:], bias2d[:])

        psum_all_cm = tc.tile_pool(name="psum_all", bufs=1, space="PSUM")
        psum_all = psum_all_cm.__enter__()
        qkv_pool = ctx.enter_context(tc.tile_pool(name="big", bufs=1))
        qkv_sb = [qkv_pool.tile([128, S_core], mm_dt, tag=f"qkvT{j}",
                                name=f"qkvT{j}") for j in range(n_mtiles)]
        rot_cm = tc.tile_pool(name="rot", bufs=1)
        rv = rot_cm.__enter__()
        rot_sb = {}
        for h in range(n_h):
            for sec in ("q", "k"):
                rot_sb[(sec, h)] = rv.tile([128, S_core], mm_dt,
                                           tag=f"rot_{sec}{h}",
                                           name=f"rot_{sec}{h}")
        RC = 1024
        rope_cm = tc.tile_pool(name="rope_scr", bufs=2)
        rope_scr = rope_cm.__enter__()

        # ------------ phase 1: qkvT = Wpack @ hidden.T --------------
        with ExitStack() as p1:
            hidden3 = hiddenT.rearrange("p (k s) -> p k s", k=NK)
            w3 = wqkvT.rearrange("p (k m) -> p k m", k=NK)
            # k-outer streaming: two psum slots hold four j-streams
            # (columns 0:512 and 512:1024), hidden tiles are tiny
            w_pool = p1.enter_context(tc.tile_pool(name="wres", bufs=1))
            w_sb = [w_pool.tile([128, dims_pad], mm_dt, tag=f"w{k}",
                                name=f"w{k}") for k in range(NK)]
            for k in range(NK):
                nc.sync.dma_start(w_sb[k][:], w3[:, k, :])
            assert n_mtiles == 4
            hid_pool = p1.enter_context(tc.tile_pool(name="hidstream",
                                                     bufs=3))
            for (h0, h1) in halves(0, S_core):
                hw = h1 - h0
                ps01 = psum_all.tile([128, BC], f32, tag="t0", name="ps01")
                ps23 = psum_all.tile([128, BC], f32, tag="t1", name="ps23")
                pj_of = lambda j: (ps01 if j < 2 else ps23,
                                   (j % 2) * 512)
                for k in range(NK):
                    ht = hid_pool.tile([128, 512], mm_dt, tag="hidc",
                                       name="hidc")
                    nc.sync.dma_start(ht[:, :hw], hidden3[:, k, h0:h1])
                    for j in range(n_mtiles):
                        psj, co = pj_of(j)
                        nc.tensor.matmul(
                            psj[:, co:co + hw],
                            r_(w_sb[k][:, j * 128:(j + 1) * 128]),
                            r_(ht[:, :hw]),
                            start=(k == 0), stop=(k == NK - 1))
                for j in range(n_mtiles):
                    psj, co = pj_of(j)
                    nc.scalar.activation(qkv_sb[j][:, h0:h1],
                                         psj[:, co:co + hw], AF.Identity,
                                         bias=bias_sb[:, j:j + 1])

        psum_all_cm.__exit__(None, None, None)
        ps_att = ctx.enter_context(tc.tile_pool(name="ps_att", bufs=1,
                                                space="PSUM"))

        # ------------ phase 2: RoPE --------------------------------
        stg = {}
        for nm in ("sa0", "sa1", "sb0", "sb1"):
            stg[nm] = rope_scr.tile([128, RC], mm_dt, tag=nm, name=nm, bufs=1)
        pair_i = 0
        for ci, f0 in enumerate(range(0, S_core, RC)):
            f1 = min(f0 + RC, S_core)
            fs = f1 - f0
            cos_sb = rope_scr.tile([128, RC], mm_dt, tag="cos", name="cos",
                                   bufs=1)
            sin_sb = rope_scr.tile([128, RC], mm_dt, tag="sin", name="sin",
                                   bufs=1)
            nc.scalar.dma_start(cos_sb[:, :fs], cosP[:, f0:f1])
            nc.scalar.dma_start(sin_sb[:, :fs], sin2P[:, f0:f1])
            if ci == 0:
                for nm in stg:
                    nc.scalar.dma_start(stg[nm][BLK:64, :], cos_sb[BLK:64, :])
            for h in range(n_h):
                for sec in ("q", "k"):
                    lo_t, lo_r = pos[(sec, h, 0)]
                    hi_t, hi_r = pos[(sec, h, 1)]
                    assert hi_t == lo_t and hi_r == lo_r + BLK
                    x = qkv_sb[lo_t]
                    dst = rot_sb[(sec, h)]
                    stga = stg[f"sa{pair_i % 2}"]
                    stgb = stg[f"sb{pair_i % 2}"]
                    nc.scalar.dma_start(stga[0:BLK, :fs],
                                        x[lo_r:lo_r + BLK, f0:f1])
                    nc.scalar.dma_start(stga[64:64 + BLK, :fs],
                                        x[hi_r:hi_r + BLK, f0:f1])
                    nc.scalar.dma_start(stgb[0:BLK, :fs],
                                        x[hi_r:hi_r + BLK, f0:f1])
                    nc.scalar.dma_start(stgb[64:64 + BLK, :fs],
                                        x[lo_r:lo_r + BLK, f0:f1])
                    nc.vector.tensor_mul(dst[0:104, f0:f1], stga[0:104, :fs],
                                         cos_sb[0:104, :fs])
                    eng = nc.gpsimd if pair_i % 2 == 0 else nc.vector
                    eng.tensor_mul(stgb[0:104, :fs], stgb[0:104, :fs],
                                   sin_sb[0:104, :fs])
                    nc.vector.tensor_add(dst[0:104, f0:f1], dst[0:104, f0:f1],
                                         stgb[0:104, :fs])
                    pair_i += 1
        rope_cm.__exit__(None, None, None)

        vaug_cm = tc.tile_pool(name="vaug", bufs=1)
        vaug_pool = vaug_cm.__enter__()
        vaug_sb = [vaug_pool.tile([128, n_tt * VW], mm_dt, tag=f"vaug{h}",
                                  name=f"vaug{h}") for h in range(n_h)]
        vinit3 = vinit.rearrange("p (t c) -> p t c", c=VW - HD)
        for h in range(n_h):
            nc.sync.dma_start(
                vaug_sb[h].rearrange("p (t c) -> p t c", c=VW)[:, :, HD:VW],
                vinit3[:, :, :])
        GRP = 4  # key tiles transposed per psum tile / copy (1 psum bank)

        def emit_vaug(h):
            gi = 0
            while gi < n_tt:
                hi_g = min(gi + GRP, n_tt)
                if all(t_tiles[g][2] - t_tiles[g][1] == 128
                       for g in range(gi, hi_g)):
                    grp = list(range(gi, hi_g))
                else:
                    grp = [gi]
                ng = len(grp)
                tp = ps_att.tile([128, GRP * HD], f32, tag="tp", name="tp")
                for x, g in enumerate(grp):
                    si, t0, t1 = t_tiles[g]
                    sz = t1 - t0
                    for half in (0, 1):
                        vt, vr = pos[("v", h, half)]
                        nc.tensor.transpose(
                            tp[:sz, x * HD + half * BLK:
                               x * HD + (half + 1) * BLK],
                            qkv_sb[vt][0:BLK, t0:t1].bitcast(f32),
                            ident[:BLK, :BLK])
                sz0 = t_tiles[grp[0]][2] - t_tiles[grp[0]][1]
                dst = vaug_sb[h].rearrange("p (t c) -> p t c", c=VW)
                src_ap = tp.rearrange("p (t c) -> p t c", c=HD)
                if h % 2 == 0:
                    nc.vector.tensor_copy(dst[:sz0, grp[0]:grp[0] + ng, 0:HD],
                                          src_ap[:sz0, 0:ng, :])
                else:
                    nc.scalar.activation(dst[:sz0, grp[0]:grp[0] + ng, 0:HD],
                                         src_ap[:sz0, 0:ng, :], AF.Identity)
                gi += ng

        # ------------ phase 4: attention ----------------------------
        attn_sb = [qkv_pool.tile([128, S_core], mm_dt, tag=f"qkvT{h}",
                                 name=f"attnT{h}") for h in range(n_h)]

        seg_ttiles = {}
        for ti, (si, t0, t1) in enumerate(t_tiles):
            seg_ttiles.setdefault(si, []).append((ti, t0, t1))

        BA = 512  # attention query-chunk width (1-bank psum slots)
        with ExitStack() as p4:
            pt_pool = p4.enter_context(tc.tile_pool(name="pt", bufs=3))
            nrm_pool = p4.enter_context(tc.tile_pool(name="nrm", bufs=2))
            unit_box = [0]

            def emit_attention(h, si, a, e):
                qT = rot_sb[("q", h)]
                kT = rot_sb[("k", h)]
                q = a
                while q < e:
                    q0, q1 = q, min(q + BA, e)
                    qs = q1 - q0
                    po = ps_att.tile([128, BA], f32,
                                     tag=f"po{unit_box[0] % 2}", name="pv")
                    tts = seg_ttiles[si]
                    for idx, (ti, t0, t1) in enumerate(tts):
                        sz = t1 - t0
                        ps = ps_att.tile([128, BA], f32, tag=f"st{idx % 2}",
                                         name="st")
                        nc.tensor.matmul(ps[:sz, :qs], r_(kT[0:104, t0:t1]),
                                         r_(qT[0:104, q0:q1]),
                                         start=True, stop=True)
                        pt = pt_pool.tile([128, BA], mm_dt, tag="pt", name="pt")
                        nc.scalar.activation(pt[:sz, :qs], ps[:sz, :qs], AF.Exp)
                        nc.tensor.matmul(
                            po[:VW, :qs],
                            r_(vaug_sb[h][:sz, ti * VW:(ti + 1) * VW]),
                            r_(pt[:sz, :qs]),
                            start=(idx == 0), stop=(idx == len(tts) - 1))
                    # partition_broadcast ucode reads physical partition 0,
                    # so shift the denominator row 96 -> 0 via DMA
                    rc = nrm_pool.tile([128, BA], f32, tag="rc", name="rc")
                    nc.vector.tensor_copy(rc[96:97, :qs], po[96:97, :qs])
                    nc.sync.dma_start(rc[0:1, :qs], rc[96:97, :qs])
                    nc.vector.reciprocal(rc[0:1, :qs], rc[0:1, :qs])
                    bc = nrm_pool.tile([128, BA], mm_dt, tag="bc", name="bc")
                    nc.gpsimd.partition_broadcast(
                        bc[0:HD, :qs], rc[0:1, :qs].bitcast(mm_dt))
                    nc.vector.tensor_mul(attn_sb[h][0:HD, q0:q1],
                                         po[0:HD, :qs], bc[0:HD, :qs])
                    unit_box[0] += 1
                    q = q1

            for h in range(n_h):
                emit_vaug(h)
            for si, (a, e) in enumerate(segs_local):
                for h in range(n_h):
                    emit_attention(h, si, a, e)

        vaug_cm.__exit__(None, None, None)
        rot_cm.__exit__(None, None, None)

        # ------------ phase 5: projection partial -------------------
        with ExitStack() as p5:
            wp_pool = p5.enter_context(tc.tile_pool(name="wp", bufs=1))
            wp_sb = []
            for kt in range(k_proj):
                t = wp_pool.tile([HD, D], mm_dt, tag=f"wp{kt}", name=f"wp{kt}")
                nc.sync.dma_start(t[:], wprojT[kt * HD:(kt + 1) * HD, :])
                wp_sb.append(t)
            out_pool = p5.enter_context(tc.tile_pool(name="outsb", bufs=3))
            for (c0, c1) in big_chunks:
                cs = c1 - c0
                for j in range(D // 128):
                    ob = out_pool.tile([128, BC], f32, tag="ob", name="ob")
                    for (h0, h1) in halves(c0, c1):
                        ps = ps_att.tile([128, 512], f32, tag=f"st{j % 2}",
                                         name="pj")
                        for kt in range(k_proj):
                            nc.tensor.matmul(
                                ps[:, :h1 - h0],
                                r_(wp_sb[kt][:, j * 128:(j + 1) * 128]),
                                r_(attn_sb[kt][0:HD, h0:h1]),
                                start=(kt == 0), stop=(kt == k_proj - 1))
                        if j % 2 == 0:
                            nc.vector.tensor_copy(ob[:, h0 - c0:h1 - c0],
                                                  ps[:, :h1 - h0])
                        else:
                            nc.scalar.activation(ob[:, h0 - c0:h1 - c0],
                                                 ps[:, :h1 - h0], AF.Identity)
                    nc.sync.dma_start(outT[j * 128:(j + 1) * 128, c0:c1],
                                      ob[:, :cs])

    nc.compile()
    return nc


def _pack_w(Wqkv, bqkv, heads, n_h, jmajor=False):
    """Per-core packed qkv weights (q rows pre-scaled).

    Returns wqkvT_tiled [128, NK*dims_pad] (k-major blocks of [128,
    dims_pad], or j-major [128, n_mtiles*NK*128] when jmajor) and bias2d
    [128, n_mtiles]."""
    pos, n_mtiles = _pack_layout(n_h)
    dims_pad = n_mtiles * 128
    W = np.zeros((dims_pad, D), np.float32)
    b = np.zeros((dims_pad,), np.float32)
    sec_off = {"q": 0, "k": D, "v": 2 * D}
    for i, h in enumerate(heads):
        for sec in ("q", "k", "v"):
            for half in (0, 1):
                t, r = pos[(sec, i, half)]
                src = sec_off[sec] + h * HD + half * BLK
                w = Wqkv[src:src + BLK, :]
                bb = bqkv[src:src + BLK]
                if sec == "q":
                    w = w * SCALE
                    bb = bb * SCALE
                W[t * 128 + r:t * 128 + r + BLK] = w
                b[t * 128 + r:t * 128 + r + BLK] = bb
    WT = np.ascontiguousarray(W.T)  # [D = NK*128, dims_pad = n_mtiles*128]
    if jmajor:
        w_tiled = np.ascontiguousarray(
            WT.reshape(NK, 128, n_mtiles, 128).transpose(1, 2, 0, 3)
            .reshape(128, n_mtiles * NK * 128))
    else:
        w_tiled = _tile_rows(WT)
    bias2d = np.ascontiguousarray(b.reshape(n_mtiles, 128).T)
    return w_tiled, bias2d


def _tile_rows(x):
    """[R, C] with R = nk*128 -> [128, nk*C] k-major tiling."""
    R, C = x.shape
    nk = R // 128
    return np.ascontiguousarray(
        x.reshape(nk, 128, C).transpose(1, 0, 2).reshape(128, nk * C))


def _pack_wproj(Wproj, heads):
    """Rows of Wproj.T for this core's head dims, stacked per head."""
    W = np.zeros((len(heads) * HD, Wproj.shape[0]), np.float32)
    for i, h in enumerate(heads):
        W[i * HD:(i + 1) * HD] = Wproj[:, h * HD:(h + 1) * HD].T
    return W


def _pack_cos_sin(cos, sin):
    """cosP/sin2P [128, S]: lo coeffs at rows 0:40, hi at 64:104, rest 0.

    sin2P row signs match rot = x*cosP + swap(x)*sin2P: lo rows hold
    -sin_lo (they multiply x_hi), hi rows hold +sin_hi (they multiply x_lo).
    """
    S = cos.shape[0]
    cosP = np.zeros((128, S), np.float32)
    sinP = np.zeros((128, S), np.float32)
    cosP[0:BLK] = cos.T[0:BLK]
    cosP[64:64 + BLK] = cos.T[BLK:HD]
    sinP[0:BLK] = -sin.T[0:BLK]
    sinP[64:64 + BLK] = sin.T[BLK:HD]
    return cosP, sinP


def _pack_cos_sin_dense(cos, sin):
    """Dense [80, S] rope coefficients for mode A.

    sin2D row signs match rot = x*cos + swap(x)*sin2D: rows 0:40 hold
    -sin_lo (they multiply x_hi), rows 40:80 hold +sin_hi (x_lo)."""
    cosT = np.ascontiguousarray(cos.T.astype(np.float32))
    sinT = sin.T.astype(np.float32)
    sin2 = np.concatenate([-sinT[0:BLK], sinT[BLK:HD]], axis=0)
    return cosT, np.ascontiguousarray(sin2)


_CACHE = {}


def kernel(hidden_states, cos, sin, Wqkv, bqkv, Wproj, bproj, cu_seqlens):
    sys.path.insert(0, "/opt/trn_rl_repo")
    from concourse import bass_utils

    hidden_states = np.asarray(hidden_states, np.float32)
    cos = np.asarray(cos, np.float32)
    sin = np.asarray(sin, np.float32)
    Wqkv = np.asarray(Wqkv, np.float32)
    bqkv = np.asarray(bqkv, np.float32)
    Wproj = np.asarray(Wproj, np.float32)
    bproj = np.asarray(bproj, np.float32)

    S, D_ = hidden_states.shape
    assert D_ == D
    segs = _segments(cu_seqlens, S)
    uniform = (S % 4 == 0) and segs == [(i * S // 4, (i + 1) * S // 4)
                                        for i in range(4)]

    hiddenT = np.ascontiguousarray(hidden_states.T)

    if uniform:
        # mode A: 2 head-groups x 4 segments, pipelined program
        n_h, S_core = H // 2, S // 4
        key = ("A", S)
        if key not in _CACHE:
            _CACHE[key] = _build_program_a(n_h, S_core)
        nc = _CACHE[key]
        import ml_dtypes
        bf = ml_dtypes.bfloat16
        cosD, sin2D = _pack_cos_sin_dense(cos, sin)
        n_tt = S_core // 128
        vinitD = np.ones((128, n_tt), np.float32)
        in_maps = []
        meta = []
        for g in range(2):
            heads = list(range(g * n_h, (g + 1) * n_h))
            wt, b2 = _pack_w(Wqkv, bqkv, heads, n_h, jmajor=True)
            wt = wt.astype(bf)
            wprojP = _tile_rows(_pack_wproj(Wproj, heads)).astype(bf)
            for s in range(4):
                sl = slice(s * S_core, (s + 1) * S_core)
                in_maps.append({
                    "hiddenT": _tile_rows(hiddenT[:, sl]).astype(bf),
                    "wqkvT": wt,
                    "bias2d": b2,
                    "cosD": np.ascontiguousarray(cosD[:, sl]),
                    "sin2D": np.ascontiguousarray(sin2D[:, sl]),
                    "wprojP": wprojP,
                    "vinitD": vinitD,
                })
                meta.append((g, s))
        res = bass_utils.run_bass_kernel_spmd(nc, in_maps,
                                              core_ids=list(range(N_CORES)))
        out = np.zeros((D, S), np.float32)
        for c, (g, s) in enumerate(meta):
            out[:, s * S_core:(s + 1) * S_core] += res.results[c]["outT"]
    else:
        # mode C: 8-way head parallel, full sequence per core
        n_h, S_core = H // N_CORES, S
        key = ("C", S, tuple(np.asarray(cu_seqlens).tolist()))
        if key not in _CACHE:
            _CACHE[key] = _build_program(n_h, S_core, segs,
                                         resident_hidden=False)
        nc = _CACHE[key]
        cosP, sin2P = _pack_cos_sin(cos, sin)

        def _vinit(segs_local):
            n_tt = sum(-(-(e - a) // 128) for a, e in segs_local)
            v = np.zeros((128, n_tt, 17), np.float32)
            v[:, :, 16] = 1.0
            return np.ascontiguousarray(v.reshape(128, n_tt * 17))

        vinit = _vinit(segs)
        hid_tiled = _tile_rows(hiddenT)
        in_maps = []
        for c in range(N_CORES):
            heads = list(range(c * n_h, (c + 1) * n_h))
            wt, b2 = _pack_w(Wqkv, bqkv, heads, n_h)
            in_maps.append({
                "hiddenT": hid_tiled,
                "wqkvT": wt,
                "bias2d": b2,
                "cosP": cosP,
                "sin2P": sin2P,
                "wprojT": _pack_wproj(Wproj, heads),
                "vinit": vinit,
            })
        res = bass_utils.run_bass_kernel_spmd(nc, in_maps,
                                              core_ids=list(range(N_CORES)))
        out = np.zeros((D, S), np.float32)
        for c in range(N_CORES):
            out += res.results[c]["outT"]

    return np.ascontiguousarray(out.T) + bproj[None, :]
